# revision 5
# baseline (speedup 1.0000x reference)
"""Causal self-attention Trainium2 kernel (B=4, T=4096, C=384, H=6).

Sharding: 8 cores = 4 batches x 2 head-groups (3 heads each). Each core
computes y_partial = attn(x[b], heads hg) @ w_proj[rows of hg]; the host
sums the two partials per batch (the "all-reduce after c_proj" done on
host during unshard).

v2: streaming-ACT design. The exp (ScalarE) stream is the bottleneck
(~230us busy); everything else is structured so ACT never stalls:
per-head PSUM accumulators, small ring buffers for P^T tiles, phase-A
projection work for tile qt+1 interleaved into tile qt's chunk stream,
causal trimming of diagonal chunks, and softmax-denominator transpose
done with K=1 matmuls on the PE instead of a DRAM round trip.
"""

import numpy as np
from contextlib import ExitStack

import concourse.bass as bass
import concourse.tile as tile
from concourse import mybir
from concourse.bass_utils import run_bass_kernel_spmd
from concourse.vector_clock import ScopedClock

F32 = mybir.dt.float32
BF16 = mybir.dt.bfloat16
EXP = mybir.ActivationFunctionType.Exp
MULT = mybir.AluOpType.mult
ADD = mybir.AluOpType.add

B, T, C, H, D = 4, 4096, 384, 6, 64
HPC = 3            # heads per core
QT = 512           # q tile
KC = 128           # key chunk
SCALE = 1.0 / 8.0  # 1/sqrt(64)


# ---------------------------------------------------------------------------
# Workaround: neuronxcc CoreV3 rejects >2 sem waits on the Tile tail drain.
# Split the drain's waits into individual sync-engine wait instructions.
def _drain_and_barrier_split(self, tick_clock, wait_clock):
    nc = self.nc
    drain_inst = nc.sync.drain()
    wait_clock.add_sem_waits(
        drain_inst.ins, ScopedClock({None: tick_clock.global_clock})
    )
    si = drain_inst.ins.sync_info
    if si is not None and si.on_wait and len(si.on_wait) > 1:
        waits = list(si.on_wait)
        si.on_wait = []
        allocated = {h.name: h for h in self.sems.allocated().values()}
        for w in waits:
            h = allocated.get(w.ant_name)
            assert h is not None, f"no sem handle for drain wait {w.ant_name}"
            assert w.wait_mode == "sem-ge-imm", w.wait_mode
            nc.sync.wait_ge(h, w.wait_value)
    nc.all_engine_barrier()
    assert self.sems is not None
    popped = nc._tile_sem_poison_stack.pop()
    assert popped is self._sem_poison
    nc.clear_and_free_semaphores(list(self.sems.allocated().values()))
    nc.all_engine_barrier()


tile.TileContext._drain_and_barrier = _drain_and_barrier_split


MAX_WAITS = 1  # CoreV3 per-instruction sem-wait capacity (S3_LW holds only 1)


def _split_excess_waits(nc):
    """Hoist sem waits beyond MAX_WAITS onto same-engine NOPs inserted
    directly before the over-limit instruction (waits are order-free)."""
    for fn in nc.m.functions:
        for bb in fn.blocks:
            insts = list(bb.instructions)
            out = []
            changed = False
            for inst in insts:
                si = inst.sync_info
                if si is not None and si.on_wait and len(si.on_wait) > MAX_WAITS:
                    waits = list(si.on_wait)
                    excess, keep = waits[:-MAX_WAITS], waits[-MAX_WAITS:]
                    si.on_wait = keep
                    inst.sync_info = si
                    for i in range(0, len(excess), MAX_WAITS):
                        nop = mybir.InstNoOp(
                            name=f"{inst.name}-waitsplit-{i}", ins=[], outs=[]
                        )
                        nop.engine = inst.engine
                        nop.sync_info = mybir.SyncInfo(
                            on_wait=excess[i:i + MAX_WAITS], on_update=[]
                        )
                        nc.register_instruction(nop)
                        out.append(nop)
                    changed = True
                out.append(inst)
            if changed:
                bb.instructions = out
# ---------------------------------------------------------------------------


def build(t=T):
    nqt = t // QT          # q tiles
    nkc = t // KC          # key chunks

    nc = bass.Bass()
    x_d = nc.dram_tensor("xT16", [C, t], BF16, kind="ExternalInput")
    wq01_d = nc.dram_tensor("w_q01", [3, 128, 128], BF16, kind="ExternalInput")
    wk01_d = nc.dram_tensor("w_k01", [3, 128, 128], BF16, kind="ExternalInput")
    # head-2 q/k weights duplicated into both column halves so the
    # projection fills both partition halves directly
    wq2_d = nc.dram_tensor("w_q2", [3, 128, 128], BF16, kind="ExternalInput")
    wk2_d = nc.dram_tensor("w_k2", [3, 128, 128], BF16, kind="ExternalInput")
    wv_d = nc.dram_tensor("w_v", [3, 128, 192], BF16, kind="ExternalInput")
    wo_d = nc.dram_tensor("w_o", [3, 64, 384], F32, kind="ExternalInput")
    tri_d = nc.dram_tensor("tri2", [128, 2, 128], BF16, kind="ExternalInput")
    ztri_d = nc.dram_tensor("ztri", [128, 256], BF16, kind="ExternalInput")
    y_d = nc.dram_tensor("y", [t, C], F32, kind="ExternalOutput")

    with tile.TileContext(nc) as tc, ExitStack() as ctx:
        persist = ctx.enter_context(tc.tile_pool(name="persist", bufs=1))

        # weights / masks
        wq01 = persist.tile([128, 3, 128], BF16)
        wk01 = persist.tile([128, 3, 128], BF16)
        wq2 = persist.tile([128, 3, 128], BF16)
        wk2 = persist.tile([128, 3, 128], BF16)
        wv = persist.tile([128, 3, 192], BF16)
        wo = persist.tile([64, 3, 384], F32)
        for c in range(3):
            nc.sync.dma_start(out=wq01[:, c, :], in_=wq01_d[c])
            nc.sync.dma_start(out=wk01[:, c, :], in_=wk01_d[c])
            nc.sync.dma_start(out=wq2[:, c, :], in_=wq2_d[c])
            nc.sync.dma_start(out=wk2[:, c, :], in_=wk2_d[c])
            nc.sync.dma_start(out=wv[:, c, :], in_=wv_d[c])
            nc.sync.dma_start(out=wo[:, c, :], in_=wo_d[c])
        tri2 = persist.tile([128, 2, 128], BF16)
        ztri = persist.tile([128, 256], BF16)
        nc.sync.dma_start(out=tri2[:], in_=tri_d[:])
        nc.sync.dma_start(out=ztri[:], in_=ztri_d[:])

        # persistent activations (bf16)
        qT01 = persist.tile([128, t], BF16)   # rows 0:64 h0 qT, 64:128 h1 qT
        kT01 = persist.tile([128, t], BF16)
        qT2 = persist.tile([128, t], BF16)    # head 2 duplicated both halves
        kT2 = persist.tile([128, t], BF16)
        vsb = persist.tile([128, nkc, 3, 65], BF16)  # [keys, chunk, head, d|one]
        ones_col = persist.tile([65, 1], F32)
        nc.vector.memset(vsb[:, :, :, 64:65], 1.0)
        nc.vector.memset(ones_col[:], 1.0)

        with (
            tc.tile_pool(name="xt", bufs=2) as xt_p,
            tc.tile_pool(name="ps", bufs=2, space="PSUM") as ps_p,
            tc.tile_pool(name="attps", bufs=1, space="PSUM") as att_p,
            tc.tile_pool(name="yps", bufs=1, space="PSUM") as y_p,
            tc.tile_pool(name="pth01", bufs=6) as pth01_p,
            tc.tile_pool(name="pth2", bufs=4) as pth2_p,
            tc.tile_pool(name="atsb", bufs=4) as at_p,
            tc.tile_pool(name="linv", bufs=6) as linv_p,
            tc.tile_pool(name="yout", bufs=3) as ysb_p,
        ):
            # ---------- phase A: projections for one 512-token block ------
            def phase_a_ops(tb):
                """Emit the x DMA immediately; return closures for the 8
                matmul+copy groups (4 q/k, then 4 v sub-blocks)."""
                xT = xt_p.tile([128, 3, QT], BF16, tag="xt", name="xT")
                for c in range(3):
                    nc.sync.dma_start(
                        out=xT[:, c, :],
                        in_=x_d[c * 128:(c + 1) * 128, tb * QT:(tb + 1) * QT],
                    )
                ops = []

                def qk_group(w_sb, dst):
                    def op():
                        ps = ps_p.tile([128, 2, QT], F32, tag="ps", name="psqk")
                        for c in range(3):
                            nc.tensor.matmul(
                                ps[:, 0, :], w_sb[:, c, :], xT[:, c, :],
                                start=(c == 0), stop=(c == 2),
                            )
                        nc.vector.tensor_copy(
                            dst[:, tb * QT:(tb + 1) * QT], ps[:, 0, :]
                        )
                    return op

                def v_group(s):
                    def op():
                        psv = ps_p.tile([128, 2, QT], F32, tag="ps", name="psv")
                        for c in range(3):
                            nc.tensor.matmul(
                                psv[:, 0, 0:192],
                                xT[:, c, s * 128:(s + 1) * 128],
                                wv[:, c, :],
                                start=(c == 0), stop=(c == 2),
                            )
                        nc.vector.tensor_copy(
                            vsb[:, tb * 4 + s, :, 0:64],
                            psv[:, 0, 0:192].rearrange("p (h d) -> p h d", h=3),
                        )
                    return op

                ops.append(qk_group(wq01, qT01))
                ops.append(qk_group(wk01, kT01))
                ops.append(qk_group(wq2, qT2))
                ops.append(qk_group(wk2, kT2))
                for s in range(4):
                    ops.append(v_group(s))
                return ops

            # prologue: block 0 projections emitted directly
            for op in phase_a_ops(0):
                op()

            for qt in range(nqt):
                nch = 4 * (qt + 1)
                q0, q1 = qt * QT, (qt + 1) * QT
                pending = phase_a_ops(qt + 1) if qt + 1 < nqt else []

                att = [
                    att_p.tile([65, QT], F32, tag=f"att{h}", name=f"att{h}")
                    for h in range(3)
                ]
                p2_prev = None
                p2_lo = 0

                for ck in range(nch):
                    j = ck - 4 * qt          # >=0 on diagonal chunks
                    lo = 128 * j if j > 0 else 0   # trimmed q start (cols)

                    # S^T h0/h1: packed pair in concurrent row groups
                    ssx = ps_p.tile([128, 2, QT], F32, tag="ps", name="ssx")
                    nc.tensor.matmul(
                        ssx[:, 0, lo:],
                        kT01[0:64, ck * KC:(ck + 1) * KC],
                        qT01[0:64, q0 + lo:q1],
                        start=True, stop=True, tile_position=(0, 0),
                    )
                    nc.tensor.matmul(
                        ssx[:, 1, lo:],
                        kT01[64:128, ck * KC:(ck + 1) * KC],
                        qT01[64:128, q0 + lo:q1],
                        start=True, stop=True, tile_position=(64, 0),
                    )
                    p01 = pth01_p.tile([128, 2, QT], BF16, tag="p01", name="p01")
                    nc.scalar.activation(
                        out=p01[:, :, lo:], in_=ssx[:, :, lo:],
                        func=EXP, scale=SCALE,
                    )
                    if j >= 0:
                        # causal mask on the 128-wide triangular block
                        nc.vector.tensor_tensor(
                            out=p01[:, :, lo:lo + 128],
                            in0=p01[:, :, lo:lo + 128],
                            in1=tri2[:, :, :], op=MULT,
                        )

                    # S^T h2: chunk pair (ck, ck+1) in concurrent row groups
                    if ck % 2 == 0:
                        jp = ck - 4 * qt     # trim to the even half's start
                        lo2 = 128 * jp if jp > 0 else 0
                        ssc = ps_p.tile([128, 2, QT], F32, tag="ps", name="ssc")
                        nc.tensor.matmul(
                            ssc[:, 0, lo2:],
                            kT2[0:64, ck * KC:(ck + 1) * KC],
                            qT2[0:64, q0 + lo2:q1],
                            start=True, stop=True, tile_position=(0, 0),
                        )
                        nc.tensor.matmul(
                            ssc[:, 1, lo2:],
                            kT2[64:128, (ck + 1) * KC:(ck + 2) * KC],
                            qT2[64:128, q0 + lo2:q1],
                            start=True, stop=True, tile_position=(64, 0),
                        )
                        p2 = pth2_p.tile([128, 2, QT], BF16, tag="p2", name="p2")
                        nc.scalar.activation(
                            out=p2[:, :, lo2:], in_=ssc[:, :, lo2:],
                            func=EXP, scale=SCALE,
                        )
                        if jp >= 0:
                            nc.vector.tensor_tensor(
                                out=p2[:, 0, lo2:lo2 + 128],
                                in0=p2[:, 0, lo2:lo2 + 128],
                                in1=tri2[:, 0, :], op=MULT,
                            )
                            nc.vector.tensor_tensor(
                                out=p2[:, 1, lo2:lo2 + 256],
                                in0=p2[:, 1, lo2:lo2 + 256],
                                in1=ztri[:, :], op=MULT,
                            )
                        p2_prev = p2
                        p2_lo = lo2

                    # att^T accumulation h0/h1 for this chunk
                    for h in range(2):
                        nc.tensor.matmul(
                            att[h][:, lo:], vsb[:, ck, h, :], p01[:, h, lo:],
                            start=(ck == 0), stop=(ck == nch - 1),
                        )
                    # att^T h2 for the completed pair
                    if ck % 2 == 1:
                        for half, cck in enumerate((ck - 1, ck)):
                            jj = cck - 4 * qt
                            hlo = 128 * jj if jj > 0 else 0
                            nc.tensor.matmul(
                                att[2][:, hlo:],
                                vsb[:, cck, 2, :],
                                p2_prev[:, half, hlo:],
                                start=(cck == 0), stop=(cck == nch - 1),
                            )

                    # interleave next block's projection work
                    if pending and ck >= 1:
                        pending.pop(0)()

                while pending:
                    pending.pop(0)()

                # ---------- epilogue: normalize + c_proj ----------
                ytile = y_p.tile([128, QT], F32, tag="y", name="ytile")
                ats = []
                for h in range(3):
                    at = at_p.tile([65, QT], F32, tag="at", name="at")
                    ats.append(at)
                    nc.vector.tensor_copy(at[:], att[h][:])
                    # transpose l row -> columns via K=1 matmuls
                    for s in range(4):
                        nc.tensor.matmul(
                            ytile[:, 384 + 4 * h + s:385 + 4 * h + s],
                            at[64:65, s * 128:(s + 1) * 128],
                            ones_col[64:65, 0:1],
                            start=True, stop=True, tile_position=(64, 0),
                        )
                linvs = []
                for h in range(3):
                    linv = linv_p.tile([128, 4], F32, tag="linv", name="linv")
                    linvs.append(linv)
                    nc.vector.reciprocal(
                        linv[:], ytile[:, 384 + 4 * h:388 + 4 * h]
                    )

                last = qt == nqt - 1
                for s in range(4):
                    ysb = ysb_p.tile([128, C], F32, tag="ysb", name="ysb")
                    if last:
                        # steal the (now idle) S^T psum ring for parallelism
                        yp_t = ps_p.tile([128, 2, QT], F32, tag="ps", name="yplast")
                        yp = yp_t[:, 0, 0:C]
                    else:
                        yp = ytile[:, 0:C]
                    for h in range(3):
                        nc.tensor.matmul(
                            yp,
                            ats[h][0:64, s * 128:(s + 1) * 128],
                            wo[:, h, :],
                            start=True, stop=True,
                        )
                        sc = linvs[h][:, s:s + 1]
                        if h == 0:
                            nc.vector.tensor_scalar(
                                out=ysb[:], in0=yp, scalar1=sc,
                                scalar2=None, op0=MULT,
                            )
                        else:
                            nc.vector.scalar_tensor_tensor(
                                out=ysb[:], in0=yp, scalar=sc, in1=ysb[:],
                                op0=MULT, op1=ADD,
                            )
                    nc.sync.dma_start(
                        out=y_d[q0 + s * 128:q0 + (s + 1) * 128, :],
                        in_=ysb[:],
                    )

    _split_excess_waits(nc)
    nc.finalize()
    return nc


_NC_CACHE = {}


def _get_nc(t=T):
    if t not in _NC_CACHE:
        _NC_CACHE[t] = build(t)
    return _NC_CACHE[t]


def _make_masks(bf16):
    f = np.arange(128)[None, :]
    p = np.arange(128)[:, None]
    tri = (f >= p).astype(np.float32)
    tri2 = np.stack([tri, tri], axis=1)               # [128, 2, 128]
    ztri = np.concatenate([np.zeros((128, 128), np.float32), tri], axis=1)
    return tri2.astype(bf16), ztri.astype(bf16)


def _prep_core_inputs(x_b, w_attn, w_proj, hg, bf16):
    """Host-side shard prep for one core: batch x_b, head group hg (0/1)."""
    h0 = 3 * hg
    q = w_attn[:, 0:C]
    k = w_attn[:, C:2 * C]
    v = w_attn[:, 2 * C:3 * C]
    qcols = lambda h: q[:, h * D:(h + 1) * D]
    kcols = lambda h: k[:, h * D:(h + 1) * D]
    w_q01 = np.concatenate([qcols(h0), qcols(h0 + 1)], axis=1)      # [384,128]
    w_k01 = np.concatenate([kcols(h0), kcols(h0 + 1)], axis=1)
    w_q2 = np.concatenate([qcols(h0 + 2), qcols(h0 + 2)], axis=1)   # dup halves
    w_k2 = np.concatenate([kcols(h0 + 2), kcols(h0 + 2)], axis=1)
    w_v = v[:, h0 * D:(h0 + 3) * D]                                 # [384,192]
    w_o = w_proj[h0 * D:(h0 + 3) * D, :]                            # [192,384]
    tri2, ztri = _make_masks(bf16)
    return {
        "xT16": np.ascontiguousarray(x_b.T, dtype=bf16),
        "w_q01": np.ascontiguousarray(w_q01.reshape(3, 128, 128), dtype=bf16),
        "w_k01": np.ascontiguousarray(w_k01.reshape(3, 128, 128), dtype=bf16),
        "w_q2": np.ascontiguousarray(w_q2.reshape(3, 128, 128), dtype=bf16),
        "w_k2": np.ascontiguousarray(w_k2.reshape(3, 128, 128), dtype=bf16),
        "w_v": np.ascontiguousarray(w_v.reshape(3, 128, 192), dtype=bf16),
        "w_o": np.ascontiguousarray(
            w_o.reshape(3, 64, 384), dtype=np.float32
        ),
        "tri2": tri2,
        "ztri": ztri,
    }


def _build_in_maps(x, w_attn, w_proj):
    import ml_dtypes
    bf16 = ml_dtypes.bfloat16
    in_maps = []
    for core in range(8):
        im = _prep_core_inputs(
            x[core // 2], w_attn, w_proj, core % 2, bf16
        )
        in_maps.append(im)
    return in_maps


def kernel(x, w_attn, w_proj):
    x = np.asarray(x, dtype=np.float32)
    w_attn = np.asarray(w_attn, dtype=np.float32)
    w_proj = np.asarray(w_proj, dtype=np.float32)
    b, t, c = x.shape

    nc = _get_nc(t)
    in_maps = _build_in_maps(x, w_attn, w_proj)

    res = run_bass_kernel_spmd(nc, in_maps, list(range(8)))
    out = np.empty((b, t, c), dtype=np.float32)
    for bb in range(b):
        out[bb] = res.results[2 * bb]["y"] + res.results[2 * bb + 1]["y"]
    return out


# revision 7
# speedup vs baseline: 1.4246x; 1.4246x over previous
"""Causal self-attention Trainium2 kernel (B=4, T=4096, C=384, H=6).

Sharding: 8 cores = 4 batches x 2 head-groups (3 heads each). Each core
computes y_partial = attn(x[b], heads hg) @ w_proj[rows of hg]; the host
sums the two partials per batch (the "all-reduce after c_proj" done on
host during unshard).

v3: streaming-ACT design. The exp (ScalarE) stream is the bottleneck
(~220us busy); everything else is structured so ACT never stalls and the
PE never loses its HAM boost clock:
- per-head PSUM accumulators + double-buffered S^T PSUM ring
- small SBUF ring buffers for P^T so cross-tile WAR never stalls exp
- phase-A projections for tile qt+1 interleaved into tile qt's stream
- causal trimming of diagonal chunks (exp/S^T/att@v only on valid cols)
- causal masking applied on the PE as an accumulated bias matmul
  (identity^T @ [-30000*tri | 0...]) so exp flushes masked cols to 0
- lowest-priority "warm keeper" dummy matmuls fill PE idle gaps so the
  HAM clock gate stays at 2.4 GHz
- a fraction of exps computed on the DVE via the Schraudolph int16
  bit-trick to offload the saturated ScalarE
"""

import numpy as np
from contextlib import ExitStack

import concourse.bass as bass
import concourse.tile as tile
from concourse import mybir
from concourse.bass_utils import run_bass_kernel_spmd
from concourse.masks import make_identity
from concourse.vector_clock import ScopedClock

F32 = mybir.dt.float32
BF16 = mybir.dt.bfloat16
I16 = mybir.dt.int16
EXP = mybir.ActivationFunctionType.Exp
MULT = mybir.AluOpType.mult
ADD = mybir.AluOpType.add

B, T, C, H, D = 4, 4096, 384, 6, 64
HPC = 3            # heads per core
QT = 512           # q tile
KC = 128           # key chunk
SCALE = 1.0 / 8.0  # 1/sqrt(64)

NEG = -30000.0     # causal bias; exp(NEG*SCALE) flushes to 0

# Schraudolph exp on DVE (int16/bf16-space): exp(x*SCALE) ~=
# bitcast_bf16(int16(x * SCH_A + SCH_B))
SCHRAUDOLPH = True
SCH_A = (128.0 / float(np.log(2.0))) * SCALE
SCH_B = 16250.5

WARM_DUMMIES = True
DUM_PER_CHUNK = 4
DUM_PER_TILE = 16
DUM_PROLOGUE = 40


# ---------------------------------------------------------------------------
# Workaround: neuronxcc CoreV3 rejects >2 sem waits on the Tile tail drain.
# Split the drain's waits into individual sync-engine wait instructions.
def _drain_and_barrier_split(self, tick_clock, wait_clock):
    nc = self.nc
    drain_inst = nc.sync.drain()
    wait_clock.add_sem_waits(
        drain_inst.ins, ScopedClock({None: tick_clock.global_clock})
    )
    si = drain_inst.ins.sync_info
    if si is not None and si.on_wait and len(si.on_wait) > 1:
        waits = list(si.on_wait)
        si.on_wait = []
        allocated = {h.name: h for h in self.sems.allocated().values()}
        for w in waits:
            h = allocated.get(w.ant_name)
            assert h is not None, f"no sem handle for drain wait {w.ant_name}"
            assert w.wait_mode == "sem-ge-imm", w.wait_mode
            nc.sync.wait_ge(h, w.wait_value)
    nc.all_engine_barrier()
    assert self.sems is not None
    popped = nc._tile_sem_poison_stack.pop()
    assert popped is self._sem_poison
    nc.clear_and_free_semaphores(list(self.sems.allocated().values()))
    nc.all_engine_barrier()


tile.TileContext._drain_and_barrier = _drain_and_barrier_split


MAX_WAITS = 1  # CoreV3 per-instruction sem-wait capacity (S3_LW holds only 1)


def _split_excess_waits(nc):
    """Hoist sem waits beyond MAX_WAITS onto same-engine NOPs inserted
    directly before the over-limit instruction (waits are order-free)."""
    for fn in nc.m.functions:
        for bb in fn.blocks:
            insts = list(bb.instructions)
            out = []
            changed = False
            for inst in insts:
                si = inst.sync_info
                if si is not None and si.on_wait and len(si.on_wait) > MAX_WAITS:
                    waits = list(si.on_wait)
                    excess, keep = waits[:-MAX_WAITS], waits[-MAX_WAITS:]
                    si.on_wait = keep
                    inst.sync_info = si
                    for i in range(0, len(excess), MAX_WAITS):
                        nop = mybir.InstNoOp(
                            name=f"{inst.name}-waitsplit-{i}", ins=[], outs=[]
                        )
                        nop.engine = inst.engine
                        nop.sync_info = mybir.SyncInfo(
                            on_wait=excess[i:i + MAX_WAITS], on_update=[]
                        )
                        nc.register_instruction(nop)
                        out.append(nop)
                    changed = True
                out.append(inst)
            if changed:
                bb.instructions = out
# ---------------------------------------------------------------------------


def build(t=T):
    nqt = t // QT          # q tiles
    nkc = t // KC          # key chunks

    nc = bass.Bass()
    x_d = nc.dram_tensor("xT16", [C, t], BF16, kind="ExternalInput")
    wq01_d = nc.dram_tensor("w_q01", [3, 128, 128], BF16, kind="ExternalInput")
    wk01_d = nc.dram_tensor("w_k01", [3, 128, 128], BF16, kind="ExternalInput")
    # head-2 q/k weights duplicated into both column halves so the
    # projection fills both partition halves directly
    wq2_d = nc.dram_tensor("w_q2", [3, 128, 128], BF16, kind="ExternalInput")
    wk2_d = nc.dram_tensor("w_k2", [3, 128, 128], BF16, kind="ExternalInput")
    wv_d = nc.dram_tensor("w_v", [3, 128, 192], BF16, kind="ExternalInput")
    wo_d = nc.dram_tensor("w_o", [3, 64, 384], BF16, kind="ExternalInput")
    bmask_d = nc.dram_tensor("bmask", [128, 640], BF16, kind="ExternalInput")
    bzmask_d = nc.dram_tensor("bzmask", [128, 768], BF16, kind="ExternalInput")
    y_d = nc.dram_tensor("y", [t, C], F32, kind="ExternalOutput")
    # scratch for transposing the softmax denominator row into columns
    l_d = nc.dram_tensor("lscratch", [t // QT, 3, QT], BF16)

    with tile.TileContext(nc) as tc, ExitStack() as ctx:
        persist = ctx.enter_context(tc.tile_pool(name="persist", bufs=1))

        # weights / masks
        wq01 = persist.tile([128, 3, 128], BF16)
        wk01 = persist.tile([128, 3, 128], BF16)
        wq2 = persist.tile([128, 3, 128], BF16)
        wk2 = persist.tile([128, 3, 128], BF16)
        wv = persist.tile([128, 3, 192], BF16)
        wo = persist.tile([64, 3, 384], BF16)
        for c in range(3):
            nc.sync.dma_start(out=wq01[:, c, :], in_=wq01_d[c])
            nc.sync.dma_start(out=wk01[:, c, :], in_=wk01_d[c])
            nc.sync.dma_start(out=wq2[:, c, :], in_=wq2_d[c])
            nc.sync.dma_start(out=wk2[:, c, :], in_=wk2_d[c])
            nc.sync.dma_start(out=wv[:, c, :], in_=wv_d[c])
            nc.sync.dma_start(out=wo[:, c, :], in_=wo_d[c])
        bmask = persist.tile([128, 640], BF16)
        bzmask = persist.tile([128, 768], BF16)
        nc.sync.dma_start(out=bmask[:], in_=bmask_d[:])
        nc.sync.dma_start(out=bzmask[:], in_=bzmask_d[:])
        ident = persist.tile([128, 128], BF16)
        make_identity(nc, ident[:])

        # persistent activations (bf16)
        qT01 = persist.tile([128, t], BF16)   # rows 0:64 h0 qT, 64:128 h1 qT
        kT01 = persist.tile([128, t], BF16)
        qT2 = persist.tile([128, t], BF16)    # head 2 duplicated both halves
        kT2 = persist.tile([128, t], BF16)
        vsb = persist.tile([128, nkc, 3, 65], BF16)  # [keys, chunk, head, d|one]
        nc.vector.memset(vsb[:, :, :, 64:65], 1.0)

        with (
            tc.tile_pool(name="xt", bufs=2) as xt_p,
            tc.tile_pool(name="ps", bufs=2, space="PSUM") as ps_p,
            tc.tile_pool(name="attps", bufs=1, space="PSUM") as att_p,
            tc.tile_pool(name="yps", bufs=1, space="PSUM") as y_p,
            tc.tile_pool(name="pth01", bufs=6) as pth01_p,
            tc.tile_pool(name="pthi", bufs=3) as pthi_p,
            tc.tile_pool(name="pth2", bufs=4) as pth2_p,
            tc.tile_pool(name="atsb", bufs=6) as at_p,
            tc.tile_pool(name="lcol", bufs=6) as lcol_p,
            tc.tile_pool(name="yout", bufs=3) as ysb_p,
        ):
            # one PSUM bank shared by c_proj output (cols 0:384) and the
            # warm-keeper dummy target (cols 384:512)
            ydum = y_p.tile([128, QT], F32, tag="y", name="ydum")

            def emit_dummies(n, rhs):
                if not WARM_DUMMIES:
                    return
                with tc.high_priority(offset=-(10 ** 9)):
                    for _ in range(n):
                        nc.tensor.matmul(
                            ydum[:, 384:512], wq01[:, 0, :], rhs,
                            start=True, stop=True,
                        )

            # ---------- phase A: projections for one 512-token block ------
            def phase_a_ops(tb):
                """Emit the x DMA immediately; return closures for the 8
                matmul+copy groups (4 q/k, then 4 v sub-blocks)."""
                xT = xt_p.tile([128, 3, QT], BF16, tag="xt", name="xT")
                for c in range(3):
                    nc.sync.dma_start(
                        out=xT[:, c, :],
                        in_=x_d[c * 128:(c + 1) * 128, tb * QT:(tb + 1) * QT],
                    )
                ops = []

                def qk_group(w_sb, dst):
                    def op():
                        ps = ps_p.tile([128, 2, QT], F32, tag="ps", name="psqk")
                        for c in range(3):
                            nc.tensor.matmul(
                                ps[:, 0, :], w_sb[:, c, :], xT[:, c, :],
                                start=(c == 0), stop=(c == 2),
                            )
                        nc.vector.tensor_copy(
                            dst[:, tb * QT:(tb + 1) * QT], ps[:, 0, :]
                        )
                    return op

                def v_group(s):
                    def op():
                        psv = ps_p.tile([128, 2, QT], F32, tag="ps", name="psv")
                        for c in range(3):
                            nc.tensor.matmul(
                                psv[:, 0, 0:192],
                                xT[:, c, s * 128:(s + 1) * 128],
                                wv[:, c, :],
                                start=(c == 0), stop=(c == 2),
                            )
                        nc.vector.tensor_copy(
                            vsb[:, tb * 4 + s, :, 0:64],
                            psv[:, 0, 0:192].rearrange(
                                "p (h d) -> p h d", h=3
                            ),
                        )
                    return op

                ops.append(qk_group(wq01, qT01))
                ops.append(qk_group(wk01, kT01))
                ops.append(qk_group(wq2, qT2))
                ops.append(qk_group(wk2, kT2))
                for s in range(4):
                    ops.append(v_group(s))
                return ops

            # prologue: warm the PE, then block-0 projections
            emit_dummies(DUM_PROLOGUE, wk01[:, 0, :])
            for op in phase_a_ops(0):
                op()

            sch_count = 0
            for qt in range(nqt):
                nch = 4 * (qt + 1)
                q0, q1 = qt * QT, (qt + 1) * QT
                pending = phase_a_ops(qt + 1) if qt + 1 < nqt else []

                att = [
                    att_p.tile([65, QT], F32, tag=f"att{h}", name=f"att{h}")
                    for h in range(3)
                ]
                p2_prev = None
                p2_lo = 0

                for ck in range(nch):
                    j = ck - 4 * qt          # >=0 on diagonal chunks
                    diag = j >= 0
                    lo = 128 * j if j > 0 else 0   # trimmed q start (cols)
                    use_sch = (
                        SCHRAUDOLPH and not diag and ck % 3 == 1
                    )

                    # S^T h0/h1: packed pair in concurrent row groups
                    ssx = ps_p.tile([128, 2, QT], F32, tag="ps", name="ssx")
                    nc.tensor.matmul(
                        ssx[:, 0, lo:],
                        kT01[0:64, ck * KC:(ck + 1) * KC],
                        qT01[0:64, q0 + lo:q1],
                        start=True, stop=not diag, tile_position=(0, 0),
                    )
                    nc.tensor.matmul(
                        ssx[:, 1, lo:],
                        kT01[64:128, ck * KC:(ck + 1) * KC],
                        qT01[64:128, q0 + lo:q1],
                        start=True, stop=not diag, tile_position=(64, 0),
                    )
                    if diag:
                        # causal bias: += ident^T @ [-30000*tri | zeros]
                        for h in range(2):
                            nc.tensor.matmul(
                                ssx[:, h, lo:], ident[:],
                                bmask[:, 0:QT - lo],
                                start=False, stop=True,
                            )
                    if use_sch:
                        p01i = pthi_p.tile(
                            [128, 2, QT], I16, tag="p01i", name="p01i"
                        )
                        nc.vector.tensor_scalar(
                            out=p01i[:, :, :], in0=ssx[:, :, :],
                            scalar1=SCH_A, scalar2=SCH_B,
                            op0=MULT, op1=ADD,
                        )
                        p01 = p01i.bitcast(BF16)
                        sch_count += 1
                    else:
                        p01 = pth01_p.tile(
                            [128, 2, QT], BF16, tag="p01", name="p01"
                        )
                        nc.scalar.activation(
                            out=p01[:, :, lo:], in_=ssx[:, :, lo:],
                            func=EXP, scale=SCALE,
                        )

                    # S^T h2: chunk pair (ck, ck+1) in concurrent row groups
                    if ck % 2 == 0:
                        jp = ck - 4 * qt     # trim to the even half's start
                        pdiag = jp >= 0
                        lo2 = 128 * jp if jp > 0 else 0
                        ssc = ps_p.tile([128, 2, QT], F32, tag="ps", name="ssc")
                        nc.tensor.matmul(
                            ssc[:, 0, lo2:],
                            kT2[0:64, ck * KC:(ck + 1) * KC],
                            qT2[0:64, q0 + lo2:q1],
                            start=True, stop=not pdiag, tile_position=(0, 0),
                        )
                        nc.tensor.matmul(
                            ssc[:, 1, lo2:],
                            kT2[64:128, (ck + 1) * KC:(ck + 2) * KC],
                            qT2[64:128, q0 + lo2:q1],
                            start=True, stop=not pdiag, tile_position=(64, 0),
                        )
                        if pdiag:
                            nc.tensor.matmul(
                                ssc[:, 0, lo2:], ident[:],
                                bmask[:, 0:QT - lo2],
                                start=False, stop=True,
                            )
                            nc.tensor.matmul(
                                ssc[:, 1, lo2:], ident[:],
                                bzmask[:, 0:QT - lo2],
                                start=False, stop=True,
                            )
                        p2 = pth2_p.tile([128, 2, QT], BF16, tag="p2", name="p2")
                        nc.scalar.activation(
                            out=p2[:, :, lo2:], in_=ssc[:, :, lo2:],
                            func=EXP, scale=SCALE,
                        )
                        p2_prev = p2
                        p2_lo = lo2

                    # att^T accumulation h0/h1 for this chunk
                    for h in range(2):
                        nc.tensor.matmul(
                            att[h][:, lo:], vsb[:, ck, h, :], p01[:, h, lo:],
                            start=(ck == 0), stop=(ck == nch - 1),
                        )
                    # att^T h2 for the completed pair
                    if ck % 2 == 1:
                        for half, cck in enumerate((ck - 1, ck)):
                            jj = cck - 4 * qt
                            hlo = 128 * jj if jj > 0 else 0
                            nc.tensor.matmul(
                                att[2][:, hlo:],
                                vsb[:, cck, 2, :],
                                p2_prev[:, half, hlo:],
                                start=(cck == 0), stop=(cck == nch - 1),
                            )

                    # interleave next block's projection work
                    if pending and ck >= 1:
                        pending.pop(0)()
                    if ck % 2 == 1:
                        emit_dummies(
                            2 * DUM_PER_CHUNK,
                            kT01[:, q0:q0 + 128],
                        )

                while pending:
                    pending.pop(0)()

                # ---------- epilogue: normalize + c_proj ----------
                ats = []
                for h in range(3):
                    at = at_p.tile([65, QT], BF16, tag="at", name="at")
                    ats.append(at)
                    nc.vector.tensor_copy(at[:], att[h][:])
                    nc.sync.dma_start(out=l_d[qt, h], in_=at[64:65, :])
                linvs = []
                for h in range(3):
                    lcol = lcol_p.tile([128, 4], BF16, tag="lcol", name="lcol")
                    nc.sync.dma_start(
                        out=lcol[:],
                        in_=l_d[qt, h].rearrange("(s p) -> p s", p=128),
                    )
                    linv = lcol_p.tile([128, 4], F32, tag="linv", name="linv")
                    linvs.append(linv)
                    nc.vector.reciprocal(linv[:], lcol[:])

                last = qt == nqt - 1
                emit_dummies(DUM_PER_TILE, kT01[:, q0:q0 + 128])
                for s in range(4):
                    ysb = ysb_p.tile([128, C], F32, tag="ysb", name="ysb")
                    if last:
                        # steal the (now idle) S^T psum ring for parallelism
                        yp_t = ps_p.tile(
                            [128, 2, QT], F32, tag="ps", name="yplast"
                        )
                        yp = yp_t[:, 0, 0:C]
                    else:
                        yp = ydum[:, 0:C]
                    for h in range(3):
                        nc.tensor.matmul(
                            yp,
                            ats[h][0:64, s * 128:(s + 1) * 128],
                            wo[:, h, :],
                            start=True, stop=True,
                        )
                        sc = linvs[h][:, s:s + 1]
                        if h == 0:
                            nc.vector.tensor_scalar(
                                out=ysb[:], in0=yp, scalar1=sc,
                                scalar2=None, op0=MULT,
                            )
                        else:
                            nc.vector.scalar_tensor_tensor(
                                out=ysb[:], in0=yp, scalar=sc, in1=ysb[:],
                                op0=MULT, op1=ADD,
                            )
                    nc.sync.dma_start(
                        out=y_d[q0 + s * 128:q0 + (s + 1) * 128, :],
                        in_=ysb[:],
                    )

    _split_excess_waits(nc)
    nc.finalize()
    return nc


_NC_CACHE = {}


def _get_nc(t=T):
    if t not in _NC_CACHE:
        _NC_CACHE[t] = build(t)
    return _NC_CACHE[t]


def _make_masks(bf16):
    f = np.arange(128)[None, :]
    p = np.arange(128)[:, None]
    tri = (f < p).astype(np.float32) * NEG     # -30000 where q < k
    zero = np.zeros((128, 512), np.float32)
    full = np.full((128, 128), NEG, np.float32)
    bmask = np.concatenate([tri, zero], axis=1)            # [128, 640]
    bzmask = np.concatenate([full, tri, zero], axis=1)     # [128, 768]
    return bmask.astype(bf16), bzmask.astype(bf16)


def _prep_core_inputs(x_b, w_attn, w_proj, hg, bf16):
    """Host-side shard prep for one core: batch x_b, head group hg (0/1)."""
    h0 = 3 * hg
    q = w_attn[:, 0:C]
    k = w_attn[:, C:2 * C]
    v = w_attn[:, 2 * C:3 * C]
    qcols = lambda h: q[:, h * D:(h + 1) * D]
    kcols = lambda h: k[:, h * D:(h + 1) * D]
    w_q01 = np.concatenate([qcols(h0), qcols(h0 + 1)], axis=1)      # [384,128]
    w_k01 = np.concatenate([kcols(h0), kcols(h0 + 1)], axis=1)
    w_q2 = np.concatenate([qcols(h0 + 2), qcols(h0 + 2)], axis=1)   # dup halves
    w_k2 = np.concatenate([kcols(h0 + 2), kcols(h0 + 2)], axis=1)
    w_v = v[:, h0 * D:(h0 + 3) * D]                                 # [384,192]
    w_o = w_proj[h0 * D:(h0 + 3) * D, :]                            # [192,384]
    bmask, bzmask = _make_masks(bf16)
    return {
        "xT16": np.ascontiguousarray(x_b.T, dtype=bf16),
        "w_q01": np.ascontiguousarray(w_q01.reshape(3, 128, 128), dtype=bf16),
        "w_k01": np.ascontiguousarray(w_k01.reshape(3, 128, 128), dtype=bf16),
        "w_q2": np.ascontiguousarray(w_q2.reshape(3, 128, 128), dtype=bf16),
        "w_k2": np.ascontiguousarray(w_k2.reshape(3, 128, 128), dtype=bf16),
        "w_v": np.ascontiguousarray(w_v.reshape(3, 128, 192), dtype=bf16),
        "w_o": np.ascontiguousarray(w_o.reshape(3, 64, 384), dtype=bf16),
        "bmask": bmask,
        "bzmask": bzmask,
    }


def _build_in_maps(x, w_attn, w_proj):
    import ml_dtypes
    bf16 = ml_dtypes.bfloat16
    in_maps = []
    for core in range(8):
        im = _prep_core_inputs(
            x[core // 2], w_attn, w_proj, core % 2, bf16
        )
        in_maps.append(im)
    return in_maps


def kernel(x, w_attn, w_proj):
    x = np.asarray(x, dtype=np.float32)
    w_attn = np.asarray(w_attn, dtype=np.float32)
    w_proj = np.asarray(w_proj, dtype=np.float32)
    b, t, c = x.shape

    nc = _get_nc(t)
    in_maps = _build_in_maps(x, w_attn, w_proj)

    res = run_bass_kernel_spmd(nc, in_maps, list(range(8)))
    out = np.empty((b, t, c), dtype=np.float32)
    for bb in range(b):
        out[bb] = res.results[2 * bb]["y"] + res.results[2 * bb + 1]["y"]
    return out


# revision 9
# speedup vs baseline: 1.4744x; 1.0350x over previous
"""Causal self-attention Trainium2 kernel (B=4, T=4096, C=384, H=6).

Sharding: 8 cores = 4 batches x 2 head-groups (3 heads each). Each core
computes y_partial = attn(x[b], heads hg) @ w_proj[rows of hg]; the host
sums the two partials per batch (the "all-reduce after c_proj" done on
host during unshard).

v3: streaming-ACT design. The exp (ScalarE) stream is the bottleneck
(~220us busy); everything else is structured so ACT never stalls and the
PE never loses its HAM boost clock:
- per-head PSUM accumulators + double-buffered S^T PSUM ring
- small SBUF ring buffers for P^T so cross-tile WAR never stalls exp
- phase-A projections for tile qt+1 interleaved into tile qt's stream
- causal trimming of diagonal chunks (exp/S^T/att@v only on valid cols)
- causal masking applied on the PE as an accumulated bias matmul
  (identity^T @ [-30000*tri | 0...]) so exp flushes masked cols to 0
- lowest-priority "warm keeper" dummy matmuls fill PE idle gaps so the
  HAM clock gate stays at 2.4 GHz
- a fraction of exps computed on the DVE via the Schraudolph int16
  bit-trick to offload the saturated ScalarE
"""

import numpy as np
from contextlib import ExitStack

import concourse.bass as bass
import concourse.tile as tile
from concourse import mybir
from concourse.bass_utils import run_bass_kernel_spmd
from concourse.masks import make_identity
from concourse.vector_clock import ScopedClock

F32 = mybir.dt.float32
BF16 = mybir.dt.bfloat16
I16 = mybir.dt.int16
EXP = mybir.ActivationFunctionType.Exp
MULT = mybir.AluOpType.mult
ADD = mybir.AluOpType.add

B, T, C, H, D = 4, 4096, 384, 6, 64
HPC = 3            # heads per core
QT = 512           # q tile
KC = 128           # key chunk
SCALE = 1.0 / 8.0  # 1/sqrt(64)

NEG = -30000.0     # causal bias; exp(NEG*SCALE) flushes to 0

# Schraudolph exp on DVE (int16/bf16-space): exp(x*SCALE) ~=
# bitcast_bf16(int16(x * SCH_A + SCH_B))
SCHRAUDOLPH = True
SCH_A = (128.0 / float(np.log(2.0))) * SCALE
SCH_B = 16250.5

WARM_DUMMIES = True
DUM_PER_CHUNK = 2
DUM_PER_TILE = 12
DUM_PROLOGUE = 40


# ---------------------------------------------------------------------------
# Workaround: neuronxcc CoreV3 rejects >2 sem waits on the Tile tail drain.
# Split the drain's waits into individual sync-engine wait instructions.
def _drain_and_barrier_split(self, tick_clock, wait_clock):
    nc = self.nc
    drain_inst = nc.sync.drain()
    wait_clock.add_sem_waits(
        drain_inst.ins, ScopedClock({None: tick_clock.global_clock})
    )
    si = drain_inst.ins.sync_info
    if si is not None and si.on_wait and len(si.on_wait) > 1:
        waits = list(si.on_wait)
        si.on_wait = []
        allocated = {h.name: h for h in self.sems.allocated().values()}
        for w in waits:
            h = allocated.get(w.ant_name)
            assert h is not None, f"no sem handle for drain wait {w.ant_name}"
            assert w.wait_mode == "sem-ge-imm", w.wait_mode
            nc.sync.wait_ge(h, w.wait_value)
    nc.all_engine_barrier()
    assert self.sems is not None
    popped = nc._tile_sem_poison_stack.pop()
    assert popped is self._sem_poison
    nc.clear_and_free_semaphores(list(self.sems.allocated().values()))
    nc.all_engine_barrier()


tile.TileContext._drain_and_barrier = _drain_and_barrier_split


MAX_WAITS = 1  # CoreV3 per-instruction sem-wait capacity (S3_LW holds only 1)


def _split_excess_waits(nc):
    """Hoist sem waits beyond MAX_WAITS onto same-engine NOPs inserted
    directly before the over-limit instruction (waits are order-free)."""
    for fn in nc.m.functions:
        for bb in fn.blocks:
            insts = list(bb.instructions)
            out = []
            changed = False
            for inst in insts:
                si = inst.sync_info
                if si is not None and si.on_wait and len(si.on_wait) > MAX_WAITS:
                    waits = list(si.on_wait)
                    excess, keep = waits[:-MAX_WAITS], waits[-MAX_WAITS:]
                    si.on_wait = keep
                    inst.sync_info = si
                    for i in range(0, len(excess), MAX_WAITS):
                        nop = mybir.InstNoOp(
                            name=f"{inst.name}-waitsplit-{i}", ins=[], outs=[]
                        )
                        nop.engine = inst.engine
                        nop.sync_info = mybir.SyncInfo(
                            on_wait=excess[i:i + MAX_WAITS], on_update=[]
                        )
                        nc.register_instruction(nop)
                        out.append(nop)
                    changed = True
                out.append(inst)
            if changed:
                bb.instructions = out
# ---------------------------------------------------------------------------


def build(t=T):
    nqt = t // QT          # q tiles
    nkc = t // KC          # key chunks

    nc = bass.Bass()
    x_d = nc.dram_tensor("xT16", [C, t], BF16, kind="ExternalInput")
    wq01_d = nc.dram_tensor("w_q01", [3, 128, 128], BF16, kind="ExternalInput")
    wk01_d = nc.dram_tensor("w_k01", [3, 128, 128], BF16, kind="ExternalInput")
    # head-2 q/k weights duplicated into both column halves so the
    # projection fills both partition halves directly
    wq2_d = nc.dram_tensor("w_q2", [3, 128, 128], BF16, kind="ExternalInput")
    wk2_d = nc.dram_tensor("w_k2", [3, 128, 128], BF16, kind="ExternalInput")
    wv_d = nc.dram_tensor("w_v", [3, 128, 192], BF16, kind="ExternalInput")
    wo_d = nc.dram_tensor("w_o", [3, 64, 384], BF16, kind="ExternalInput")
    bmask_d = nc.dram_tensor("bmask", [128, 640], BF16, kind="ExternalInput")
    bzmask_d = nc.dram_tensor("bzmask", [128, 768], BF16, kind="ExternalInput")
    y_d = nc.dram_tensor("y", [t, C], F32, kind="ExternalOutput")
    # scratch for transposing the softmax denominator row into columns
    l_d = nc.dram_tensor("lscratch", [t // QT, 3, QT], BF16)

    with tile.TileContext(nc) as tc, ExitStack() as ctx:
        persist = ctx.enter_context(tc.tile_pool(name="persist", bufs=1))

        # weights / masks
        wq01 = persist.tile([128, 3, 128], BF16)
        wk01 = persist.tile([128, 3, 128], BF16)
        wq2 = persist.tile([128, 3, 128], BF16)
        wk2 = persist.tile([128, 3, 128], BF16)
        wv = persist.tile([128, 3, 192], BF16)
        wo = persist.tile([64, 3, 384], BF16)
        for c in range(3):
            nc.sync.dma_start(out=wq01[:, c, :], in_=wq01_d[c])
            nc.sync.dma_start(out=wk01[:, c, :], in_=wk01_d[c])
            nc.sync.dma_start(out=wq2[:, c, :], in_=wq2_d[c])
            nc.sync.dma_start(out=wk2[:, c, :], in_=wk2_d[c])
            nc.sync.dma_start(out=wv[:, c, :], in_=wv_d[c])
            nc.sync.dma_start(out=wo[:, c, :], in_=wo_d[c])
        bmask = persist.tile([128, 640], BF16)
        bzmask = persist.tile([128, 768], BF16)
        nc.sync.dma_start(out=bmask[:], in_=bmask_d[:])
        nc.sync.dma_start(out=bzmask[:], in_=bzmask_d[:])
        ident = persist.tile([128, 128], BF16)
        make_identity(nc, ident[:])

        # persistent activations (bf16)
        qT01 = persist.tile([128, t], BF16)   # rows 0:64 h0 qT, 64:128 h1 qT
        kT01 = persist.tile([128, t], BF16)
        qT2 = persist.tile([128, t], BF16)    # head 2 duplicated both halves
        kT2 = persist.tile([128, t], BF16)
        vsb = persist.tile([128, nkc, 3, 65], BF16)  # [keys, chunk, head, d|one]
        nc.vector.memset(vsb[:, :, :, 64:65], 1.0)

        with (
            tc.tile_pool(name="xt", bufs=2) as xt_p,
            tc.tile_pool(name="ps", bufs=2, space="PSUM") as ps_p,
            tc.tile_pool(name="attps", bufs=1, space="PSUM") as att_p,
            tc.tile_pool(name="yps", bufs=1, space="PSUM") as y_p,
            tc.tile_pool(name="pth01", bufs=6) as pth01_p,
            tc.tile_pool(name="pthi", bufs=3) as pthi_p,
            tc.tile_pool(name="pth2", bufs=4) as pth2_p,
            tc.tile_pool(name="atsb", bufs=6) as at_p,
            tc.tile_pool(name="lcol", bufs=6) as lcol_p,
            tc.tile_pool(name="yout", bufs=3) as ysb_p,
        ):
            # one PSUM bank shared by c_proj output (cols 0:384) and the
            # warm-keeper dummy target (cols 384:512)
            ydum = y_p.tile([128, QT], F32, tag="y", name="ydum")

            def emit_dummies(n, rhs):
                if not WARM_DUMMIES:
                    return
                with tc.high_priority(offset=-(10 ** 9)):
                    for _ in range(n):
                        nc.tensor.matmul(
                            ydum[:, 384:512], wq01[:, 0, :], rhs,
                            start=True, stop=True,
                        )

            # ---------- phase A: projections for one 512-token block ------
            def phase_a_ops(tb):
                """Emit the x DMA immediately; return closures for the
                projection groups: 2 consolidated q/k pair groups on the
                S^T psum ring + 4 v sub-blocks accumulated in the ydum
                bank (zero ring pressure)."""
                xT = xt_p.tile([128, 3, QT], BF16, tag="xt", name="xT")
                for c in range(3):
                    nc.sync.dma_start(
                        out=xT[:, c, :],
                        in_=x_d[c * 128:(c + 1) * 128, tb * QT:(tb + 1) * QT],
                    )
                ops = []

                def qk_pair(wa_sb, wb_sb, dsta, dstb):
                    def op():
                        ps = ps_p.tile([128, 2, QT], F32, tag="ps", name="psqk")
                        for half, w_sb in ((0, wa_sb), (1, wb_sb)):
                            for c in range(3):
                                nc.tensor.matmul(
                                    ps[:, half, :], w_sb[:, c, :], xT[:, c, :],
                                    start=(c == 0), stop=(c == 2),
                                )
                        nc.vector.tensor_copy(
                            dsta[:, tb * QT:(tb + 1) * QT], ps[:, 0, :]
                        )
                        nc.vector.tensor_copy(
                            dstb[:, tb * QT:(tb + 1) * QT], ps[:, 1, :]
                        )
                    return op

                def v_group(s):
                    def op():
                        for c in range(3):
                            nc.tensor.matmul(
                                ydum[:, 0:192],
                                xT[:, c, s * 128:(s + 1) * 128],
                                wv[:, c, :],
                                start=(c == 0), stop=(c == 2),
                            )
                        nc.vector.tensor_copy(
                            vsb[:, tb * 4 + s, :, 0:64],
                            ydum[:, 0:192].rearrange(
                                "p (h d) -> p h d", h=3
                            ),
                        )
                    return op

                ops.append(qk_pair(wq01, wk01, qT01, kT01))
                ops.append(qk_pair(wq2, wk2, qT2, kT2))
                for s in range(4):
                    ops.append(v_group(s))
                return ops

            # prologue: warm the PE, then block-0 projections
            emit_dummies(DUM_PROLOGUE, wk01[:, 0, :])
            for op in phase_a_ops(0):
                op()

            sch_count = 0
            for qt in range(nqt):
                nch = 4 * (qt + 1)
                q0, q1 = qt * QT, (qt + 1) * QT
                pending = phase_a_ops(qt + 1) if qt + 1 < nqt else []

                att = [
                    att_p.tile([65, QT], F32, tag=f"att{h}", name=f"att{h}")
                    for h in range(3)
                ]
                p2_prev = None
                p2_lo = 0

                for ck in range(nch):
                    j = ck - 4 * qt          # >=0 on diagonal chunks
                    diag = j >= 0
                    lo = 128 * j if j > 0 else 0   # trimmed q start (cols)
                    use_sch = (
                        SCHRAUDOLPH and not diag and ck % 3 == 1
                    )

                    # S^T h0/h1: packed pair in concurrent row groups
                    ssx = ps_p.tile([128, 2, QT], F32, tag="ps", name="ssx")
                    nc.tensor.matmul(
                        ssx[:, 0, lo:],
                        kT01[0:64, ck * KC:(ck + 1) * KC],
                        qT01[0:64, q0 + lo:q1],
                        start=True, stop=not diag, tile_position=(0, 0),
                    )
                    nc.tensor.matmul(
                        ssx[:, 1, lo:],
                        kT01[64:128, ck * KC:(ck + 1) * KC],
                        qT01[64:128, q0 + lo:q1],
                        start=True, stop=not diag, tile_position=(64, 0),
                    )
                    if diag:
                        # causal bias: += ident^T @ [-30000*tri | zeros]
                        for h in range(2):
                            nc.tensor.matmul(
                                ssx[:, h, lo:], ident[:],
                                bmask[:, 0:QT - lo],
                                start=False, stop=True,
                            )
                    if use_sch:
                        p01i = pthi_p.tile(
                            [128, 2, QT], I16, tag="p01i", name="p01i"
                        )
                        nc.vector.tensor_scalar(
                            out=p01i[:, :, :], in0=ssx[:, :, :],
                            scalar1=SCH_A, scalar2=SCH_B,
                            op0=MULT, op1=ADD,
                        )
                        p01 = p01i.bitcast(BF16)
                        sch_count += 1
                    else:
                        p01 = pth01_p.tile(
                            [128, 2, QT], BF16, tag="p01", name="p01"
                        )
                        nc.scalar.activation(
                            out=p01[:, :, lo:], in_=ssx[:, :, lo:],
                            func=EXP, scale=SCALE,
                        )

                    # S^T h2: chunk pair (ck, ck+1) in concurrent row groups
                    if ck % 2 == 0:
                        jp = ck - 4 * qt     # trim to the even half's start
                        pdiag = jp >= 0
                        lo2 = 128 * jp if jp > 0 else 0
                        ssc = ps_p.tile([128, 2, QT], F32, tag="ps", name="ssc")
                        nc.tensor.matmul(
                            ssc[:, 0, lo2:],
                            kT2[0:64, ck * KC:(ck + 1) * KC],
                            qT2[0:64, q0 + lo2:q1],
                            start=True, stop=not pdiag, tile_position=(0, 0),
                        )
                        nc.tensor.matmul(
                            ssc[:, 1, lo2:],
                            kT2[64:128, (ck + 1) * KC:(ck + 2) * KC],
                            qT2[64:128, q0 + lo2:q1],
                            start=True, stop=not pdiag, tile_position=(64, 0),
                        )
                        if pdiag:
                            nc.tensor.matmul(
                                ssc[:, 0, lo2:], ident[:],
                                bmask[:, 0:QT - lo2],
                                start=False, stop=True,
                            )
                            nc.tensor.matmul(
                                ssc[:, 1, lo2:], ident[:],
                                bzmask[:, 0:QT - lo2],
                                start=False, stop=True,
                            )
                        p2 = pth2_p.tile([128, 2, QT], BF16, tag="p2", name="p2")
                        nc.scalar.activation(
                            out=p2[:, :, lo2:], in_=ssc[:, :, lo2:],
                            func=EXP, scale=SCALE,
                        )
                        p2_prev = p2
                        p2_lo = lo2

                    # att^T accumulation h0/h1 for this chunk
                    for h in range(2):
                        nc.tensor.matmul(
                            att[h][:, lo:], vsb[:, ck, h, :], p01[:, h, lo:],
                            start=(ck == 0), stop=(ck == nch - 1),
                        )
                    # att^T h2 for the completed pair
                    if ck % 2 == 1:
                        for half, cck in enumerate((ck - 1, ck)):
                            jj = cck - 4 * qt
                            hlo = 128 * jj if jj > 0 else 0
                            nc.tensor.matmul(
                                att[2][:, hlo:],
                                vsb[:, cck, 2, :],
                                p2_prev[:, half, hlo:],
                                start=(cck == 0), stop=(cck == nch - 1),
                            )

                    # interleave next block's projection work
                    if pending and ck >= 1:
                        pending.pop(0)()
                    if ck % 2 == 1:
                        emit_dummies(
                            2 * DUM_PER_CHUNK,
                            kT01[:, q0:q0 + 128],
                        )

                while pending:
                    pending.pop(0)()

                # ---------- epilogue: normalize + c_proj ----------
                ats = []
                for h in range(3):
                    at = at_p.tile([65, QT], BF16, tag="at", name="at")
                    ats.append(at)
                    nc.vector.tensor_copy(at[:], att[h][:])
                    nc.sync.dma_start(out=l_d[qt, h], in_=at[64:65, :])
                linvs = []
                for h in range(3):
                    lcol = lcol_p.tile([128, 4], BF16, tag="lcol", name="lcol")
                    nc.sync.dma_start(
                        out=lcol[:],
                        in_=l_d[qt, h].rearrange("(s p) -> p s", p=128),
                    )
                    linv = lcol_p.tile([128, 4], F32, tag="linv", name="linv")
                    linvs.append(linv)
                    nc.vector.reciprocal(linv[:], lcol[:])

                last = qt == nqt - 1
                emit_dummies(DUM_PER_TILE, kT01[:, q0:q0 + 128])
                for s in range(4):
                    ysb = ysb_p.tile([128, C], F32, tag="ysb", name="ysb")
                    if last:
                        # steal the (now idle) S^T psum ring for parallelism
                        yp_t = ps_p.tile(
                            [128, 2, QT], F32, tag="ps", name="yplast"
                        )
                        yp = yp_t[:, 0, 0:C]
                    else:
                        yp = ydum[:, 0:C]
                    for h in range(3):
                        nc.tensor.matmul(
                            yp,
                            ats[h][0:64, s * 128:(s + 1) * 128],
                            wo[:, h, :],
                            start=True, stop=True,
                        )
                        sc = linvs[h][:, s:s + 1]
                        if h == 0:
                            nc.vector.tensor_scalar(
                                out=ysb[:], in0=yp, scalar1=sc,
                                scalar2=None, op0=MULT,
                            )
                        else:
                            nc.vector.scalar_tensor_tensor(
                                out=ysb[:], in0=yp, scalar=sc, in1=ysb[:],
                                op0=MULT, op1=ADD,
                            )
                    nc.sync.dma_start(
                        out=y_d[q0 + s * 128:q0 + (s + 1) * 128, :],
                        in_=ysb[:],
                    )

    _split_excess_waits(nc)
    nc.finalize()
    return nc


_NC_CACHE = {}


def _get_nc(t=T):
    if t not in _NC_CACHE:
        _NC_CACHE[t] = build(t)
    return _NC_CACHE[t]


def _make_masks(bf16):
    f = np.arange(128)[None, :]
    p = np.arange(128)[:, None]
    tri = (f < p).astype(np.float32) * NEG     # -30000 where q < k
    zero = np.zeros((128, 512), np.float32)
    full = np.full((128, 128), NEG, np.float32)
    bmask = np.concatenate([tri, zero], axis=1)            # [128, 640]
    bzmask = np.concatenate([full, tri, zero], axis=1)     # [128, 768]
    return bmask.astype(bf16), bzmask.astype(bf16)


def _prep_core_inputs(x_b, w_attn, w_proj, hg, bf16):
    """Host-side shard prep for one core: batch x_b, head group hg (0/1)."""
    h0 = 3 * hg
    q = w_attn[:, 0:C]
    k = w_attn[:, C:2 * C]
    v = w_attn[:, 2 * C:3 * C]
    qcols = lambda h: q[:, h * D:(h + 1) * D]
    kcols = lambda h: k[:, h * D:(h + 1) * D]
    w_q01 = np.concatenate([qcols(h0), qcols(h0 + 1)], axis=1)      # [384,128]
    w_k01 = np.concatenate([kcols(h0), kcols(h0 + 1)], axis=1)
    w_q2 = np.concatenate([qcols(h0 + 2), qcols(h0 + 2)], axis=1)   # dup halves
    w_k2 = np.concatenate([kcols(h0 + 2), kcols(h0 + 2)], axis=1)
    w_v = v[:, h0 * D:(h0 + 3) * D]                                 # [384,192]
    w_o = w_proj[h0 * D:(h0 + 3) * D, :]                            # [192,384]
    bmask, bzmask = _make_masks(bf16)
    return {
        "xT16": np.ascontiguousarray(x_b.T, dtype=bf16),
        "w_q01": np.ascontiguousarray(w_q01.reshape(3, 128, 128), dtype=bf16),
        "w_k01": np.ascontiguousarray(w_k01.reshape(3, 128, 128), dtype=bf16),
        "w_q2": np.ascontiguousarray(w_q2.reshape(3, 128, 128), dtype=bf16),
        "w_k2": np.ascontiguousarray(w_k2.reshape(3, 128, 128), dtype=bf16),
        "w_v": np.ascontiguousarray(w_v.reshape(3, 128, 192), dtype=bf16),
        "w_o": np.ascontiguousarray(w_o.reshape(3, 64, 384), dtype=bf16),
        "bmask": bmask,
        "bzmask": bzmask,
    }


def _build_in_maps(x, w_attn, w_proj):
    import ml_dtypes
    bf16 = ml_dtypes.bfloat16
    in_maps = []
    for core in range(8):
        im = _prep_core_inputs(
            x[core // 2], w_attn, w_proj, core % 2, bf16
        )
        in_maps.append(im)
    return in_maps


def kernel(x, w_attn, w_proj):
    x = np.asarray(x, dtype=np.float32)
    w_attn = np.asarray(w_attn, dtype=np.float32)
    w_proj = np.asarray(w_proj, dtype=np.float32)
    b, t, c = x.shape

    nc = _get_nc(t)
    in_maps = _build_in_maps(x, w_attn, w_proj)

    res = run_bass_kernel_spmd(nc, in_maps, list(range(8)))
    out = np.empty((b, t, c), dtype=np.float32)
    for bb in range(b):
        out[bb] = res.results[2 * bb]["y"] + res.results[2 * bb + 1]["y"]
    return out


# revision 14
# speedup vs baseline: 1.4803x; 1.0040x over previous
"""Causal self-attention Trainium2 kernel (B=4, T=4096, C=384, H=6).

Sharding: 8 cores = 4 batches x 2 head-groups (3 heads each). Each core
computes y_partial = attn(x[b], heads hg) @ w_proj[rows of hg]; the host
sums the two partials per batch (the "all-reduce after c_proj" done on
host during unshard).

v3: streaming-ACT design. The exp (ScalarE) stream is the bottleneck
(~220us busy); everything else is structured so ACT never stalls and the
PE never loses its HAM boost clock:
- per-head PSUM accumulators + double-buffered S^T PSUM ring
- small SBUF ring buffers for P^T so cross-tile WAR never stalls exp
- phase-A projections for tile qt+1 interleaved into tile qt's stream
- causal trimming of diagonal chunks (exp/S^T/att@v only on valid cols)
- causal masking applied on the PE as an accumulated bias matmul
  (identity^T @ [-30000*tri | 0...]) so exp flushes masked cols to 0
- lowest-priority "warm keeper" dummy matmuls fill PE idle gaps so the
  HAM clock gate stays at 2.4 GHz
- a fraction of exps computed on the DVE via the Schraudolph int16
  bit-trick to offload the saturated ScalarE
"""

import numpy as np
from contextlib import ExitStack

import concourse.bass as bass
import concourse.tile as tile
from concourse import mybir
from concourse.bass_utils import run_bass_kernel_spmd
from concourse.masks import make_identity
from concourse.vector_clock import ScopedClock

F32 = mybir.dt.float32
BF16 = mybir.dt.bfloat16
I16 = mybir.dt.int16
EXP = mybir.ActivationFunctionType.Exp
MULT = mybir.AluOpType.mult
ADD = mybir.AluOpType.add

B, T, C, H, D = 4, 4096, 384, 6, 64
HPC = 3            # heads per core
QT = 512           # q tile
KC = 128           # key chunk
SCALE = 1.0 / 8.0  # 1/sqrt(64)

NEG = -30000.0     # causal bias; exp(NEG*SCALE) flushes to 0

# Schraudolph exp on DVE (int16/bf16-space): exp(x*SCALE) ~=
# bitcast_bf16(int16(x * SCH_A + SCH_B))
SCHRAUDOLPH = True
SCH_A = (128.0 / float(np.log(2.0))) * SCALE
SCH_B = 16250.5

WARM_DUMMIES = True
DUM_PER_CHUNK = 2
DUM_PER_TILE = 12
DUM_PROLOGUE = 40


# ---------------------------------------------------------------------------
# Workaround: neuronxcc CoreV3 rejects >2 sem waits on the Tile tail drain.
# Split the drain's waits into individual sync-engine wait instructions.
def _drain_and_barrier_split(self, tick_clock, wait_clock):
    nc = self.nc
    drain_inst = nc.sync.drain()
    wait_clock.add_sem_waits(
        drain_inst.ins, ScopedClock({None: tick_clock.global_clock})
    )
    si = drain_inst.ins.sync_info
    if si is not None and si.on_wait and len(si.on_wait) > 1:
        waits = list(si.on_wait)
        si.on_wait = []
        allocated = {h.name: h for h in self.sems.allocated().values()}
        for w in waits:
            h = allocated.get(w.ant_name)
            assert h is not None, f"no sem handle for drain wait {w.ant_name}"
            assert w.wait_mode == "sem-ge-imm", w.wait_mode
            nc.sync.wait_ge(h, w.wait_value)
    nc.all_engine_barrier()
    assert self.sems is not None
    popped = nc._tile_sem_poison_stack.pop()
    assert popped is self._sem_poison
    nc.clear_and_free_semaphores(list(self.sems.allocated().values()))
    nc.all_engine_barrier()


tile.TileContext._drain_and_barrier = _drain_and_barrier_split


MAX_WAITS = 1  # CoreV3 per-instruction sem-wait capacity (S3_LW holds only 1)


def _split_excess_waits(nc):
    """Hoist sem waits beyond MAX_WAITS onto same-engine NOPs inserted
    directly before the over-limit instruction (waits are order-free)."""
    for fn in nc.m.functions:
        for bb in fn.blocks:
            insts = list(bb.instructions)
            out = []
            changed = False
            for inst in insts:
                si = inst.sync_info
                if si is not None and si.on_wait and len(si.on_wait) > MAX_WAITS:
                    waits = list(si.on_wait)
                    excess, keep = waits[:-MAX_WAITS], waits[-MAX_WAITS:]
                    si.on_wait = keep
                    inst.sync_info = si
                    for i in range(0, len(excess), MAX_WAITS):
                        nop = mybir.InstNoOp(
                            name=f"{inst.name}-waitsplit-{i}", ins=[], outs=[]
                        )
                        nop.engine = inst.engine
                        nop.sync_info = mybir.SyncInfo(
                            on_wait=excess[i:i + MAX_WAITS], on_update=[]
                        )
                        nc.register_instruction(nop)
                        out.append(nop)
                    changed = True
                out.append(inst)
            if changed:
                bb.instructions = out
# ---------------------------------------------------------------------------


def build(t=T):
    nqt = t // QT          # q tiles
    nkc = t // KC          # key chunks

    nc = bass.Bass()
    x_d = nc.dram_tensor("xT16", [C, t], BF16, kind="ExternalInput")
    wq01_d = nc.dram_tensor("w_q01", [3, 128, 128], BF16, kind="ExternalInput")
    wk01_d = nc.dram_tensor("w_k01", [3, 128, 128], BF16, kind="ExternalInput")
    # head-2 q/k weights duplicated into both column halves so the
    # projection fills both partition halves directly
    wq2_d = nc.dram_tensor("w_q2", [3, 128, 128], BF16, kind="ExternalInput")
    wk2_d = nc.dram_tensor("w_k2", [3, 128, 128], BF16, kind="ExternalInput")
    wv_d = nc.dram_tensor("w_v", [3, 128, 192], BF16, kind="ExternalInput")
    wo_d = nc.dram_tensor("w_o", [3, 64, 384], BF16, kind="ExternalInput")
    bmask_d = nc.dram_tensor("bmask", [128, 640], BF16, kind="ExternalInput")
    bzmask_d = nc.dram_tensor("bzmask", [128, 768], BF16, kind="ExternalInput")
    y_d = nc.dram_tensor("y", [t, C], F32, kind="ExternalOutput")
    # scratch for transposing the softmax denominator row into columns
    l_d = nc.dram_tensor("lscratch", [t // QT, 3, QT], BF16)

    with tile.TileContext(nc) as tc, ExitStack() as ctx:
        persist = ctx.enter_context(tc.tile_pool(name="persist", bufs=1))

        # weights / masks
        wq01 = persist.tile([128, 3, 128], BF16)
        wk01 = persist.tile([128, 3, 128], BF16)
        wq2 = persist.tile([128, 3, 128], BF16)
        wk2 = persist.tile([128, 3, 128], BF16)
        wv = persist.tile([128, 3, 192], BF16)
        wo = persist.tile([64, 3, 384], BF16)
        for c in range(3):
            nc.sync.dma_start(out=wq01[:, c, :], in_=wq01_d[c])
            nc.sync.dma_start(out=wk01[:, c, :], in_=wk01_d[c])
            nc.sync.dma_start(out=wq2[:, c, :], in_=wq2_d[c])
            nc.sync.dma_start(out=wk2[:, c, :], in_=wk2_d[c])
            nc.sync.dma_start(out=wv[:, c, :], in_=wv_d[c])
            nc.sync.dma_start(out=wo[:, c, :], in_=wo_d[c])
        bmask = persist.tile([128, 640], BF16)
        bzmask = persist.tile([128, 768], BF16)
        nc.sync.dma_start(out=bmask[:], in_=bmask_d[:])
        nc.sync.dma_start(out=bzmask[:], in_=bzmask_d[:])
        ident = persist.tile([128, 128], BF16)
        make_identity(nc, ident[:])

        # persistent activations (bf16)
        qT01 = persist.tile([128, t], BF16)   # rows 0:64 h0 qT, 64:128 h1 qT
        kT01 = persist.tile([128, t], BF16)
        qT2 = persist.tile([128, t], BF16)    # head 2 duplicated both halves
        kT2 = persist.tile([128, t], BF16)
        vsb = persist.tile([128, nkc, 3, 65], BF16)  # [keys, chunk, head, d|one]
        nc.vector.memset(vsb[:, :, :, 64:65], 1.0)

        with (
            tc.tile_pool(name="xt", bufs=2) as xt_p,
            tc.tile_pool(name="ps", bufs=2, space="PSUM") as ps_p,
            tc.tile_pool(name="attps", bufs=1, space="PSUM") as att_p,
            tc.tile_pool(name="yps", bufs=1, space="PSUM") as y_p,
            tc.tile_pool(name="pth01", bufs=6) as pth01_p,
            tc.tile_pool(name="pthi", bufs=3) as pthi_p,
            tc.tile_pool(name="pth2", bufs=4) as pth2_p,
            tc.tile_pool(name="atsb", bufs=6) as at_p,
            tc.tile_pool(name="lcol", bufs=6) as lcol_p,
            tc.tile_pool(name="yout", bufs=3) as ysb_p,
        ):
            # one PSUM bank shared by c_proj output (cols 0:384) and the
            # warm-keeper dummy target (cols 384:512)
            ydum = y_p.tile([128, QT], F32, tag="y", name="ydum")

            def emit_dummies(n, rhs):
                if not WARM_DUMMIES:
                    return
                with tc.high_priority(offset=-(10 ** 9)):
                    for _ in range(n):
                        nc.tensor.matmul(
                            ydum[:, 384:512], wq01[:, 0, :], rhs,
                            start=True, stop=True,
                        )

            # ---------- phase A: projections for one 512-token block ------
            def phase_a_ops(tb):
                """Emit the x DMA immediately; return closures for the
                projection groups: 2 consolidated q/k pair groups on the
                S^T psum ring + 4 v sub-blocks accumulated in the ydum
                bank (zero ring pressure)."""
                xT = xt_p.tile([128, 3, QT], BF16, tag="xt", name="xT")
                for c in range(3):
                    nc.sync.dma_start(
                        out=xT[:, c, :],
                        in_=x_d[c * 128:(c + 1) * 128, tb * QT:(tb + 1) * QT],
                    )
                ops = []

                def qk_group(w_sb, dst):
                    def op():
                        ps = ps_p.tile([128, 2, QT], F32, tag="ps", name="psqk")
                        for c in range(3):
                            nc.tensor.matmul(
                                ps[:, 0, :], w_sb[:, c, :], xT[:, c, :],
                                start=(c == 0), stop=(c == 2),
                            )
                        nc.vector.tensor_copy(
                            dst[:, tb * QT:(tb + 1) * QT], ps[:, 0, :]
                        )
                    return op

                def v_group(s):
                    def op():
                        for c in range(3):
                            nc.tensor.matmul(
                                ydum[:, 0:192],
                                xT[:, c, s * 128:(s + 1) * 128],
                                wv[:, c, :],
                                start=(c == 0), stop=(c == 2),
                            )
                        nc.vector.tensor_copy(
                            vsb[:, tb * 4 + s, :, 0:64],
                            ydum[:, 0:192].rearrange(
                                "p (h d) -> p h d", h=3
                            ),
                        )
                    return op

                ops.append(qk_group(wq01, qT01))
                ops.append(qk_group(wk01, kT01))
                ops.append(qk_group(wq2, qT2))
                ops.append(qk_group(wk2, kT2))
                for s in range(4):
                    ops.append(v_group(s))
                return ops

            # prologue: warm the PE, then block-0 projections
            emit_dummies(DUM_PROLOGUE, wk01[:, 0, :])
            for op in phase_a_ops(0):
                op()

            sch_count = 0
            for qt in range(nqt):
                nch = 4 * (qt + 1)
                q0, q1 = qt * QT, (qt + 1) * QT
                pending = phase_a_ops(qt + 1) if qt + 1 < nqt else []

                att = [
                    att_p.tile([65, QT], F32, tag=f"att{h}", name=f"att{h}")
                    for h in range(3)
                ]

                def st_h01(ck):
                    """S^T + exp (or Schraudolph) for heads 0/1, one chunk.
                    Returns the bf16 P^T view [128, 2, QT]."""
                    j = ck - 4 * qt
                    diag = j >= 0
                    lo = 128 * j if j > 0 else 0
                    use_sch = SCHRAUDOLPH and not diag and ck % 3 == 1
                    ssx = ps_p.tile([128, 2, QT], F32, tag="ps", name="ssx")
                    nc.tensor.matmul(
                        ssx[:, 0, lo:],
                        kT01[0:64, ck * KC:(ck + 1) * KC],
                        qT01[0:64, q0 + lo:q1],
                        start=True, stop=not diag, tile_position=(0, 0),
                    )
                    nc.tensor.matmul(
                        ssx[:, 1, lo:],
                        kT01[64:128, ck * KC:(ck + 1) * KC],
                        qT01[64:128, q0 + lo:q1],
                        start=True, stop=not diag, tile_position=(64, 0),
                    )
                    if diag:
                        # causal bias: += ident^T @ [-30000*tri | zeros]
                        for h in range(2):
                            nc.tensor.matmul(
                                ssx[:, h, lo:], ident[:],
                                bmask[:, 0:QT - lo],
                                start=False, stop=True,
                            )
                    if use_sch:
                        p01i = pthi_p.tile(
                            [128, 2, QT], I16, tag="p01i", name="p01i"
                        )
                        nc.vector.tensor_scalar(
                            out=p01i[:, :, :], in0=ssx[:, :, :],
                            scalar1=SCH_A, scalar2=SCH_B,
                            op0=MULT, op1=ADD,
                        )
                        return p01i.bitcast(BF16), lo
                    p01 = pth01_p.tile(
                        [128, 2, QT], BF16, tag="p01", name="p01"
                    )
                    nc.scalar.activation(
                        out=p01[:, :, lo:], in_=ssx[:, :, lo:],
                        func=EXP, scale=SCALE,
                    )
                    return p01, lo

                def st_h2(ck):
                    """S^T + exp for head 2, chunk pair (ck, ck+1) packed in
                    concurrent row groups."""
                    jp = ck - 4 * qt
                    pdiag = jp >= 0
                    lo2 = 128 * jp if jp > 0 else 0
                    ssc = ps_p.tile([128, 2, QT], F32, tag="ps", name="ssc")
                    nc.tensor.matmul(
                        ssc[:, 0, lo2:],
                        kT2[0:64, ck * KC:(ck + 1) * KC],
                        qT2[0:64, q0 + lo2:q1],
                        start=True, stop=not pdiag, tile_position=(0, 0),
                    )
                    nc.tensor.matmul(
                        ssc[:, 1, lo2:],
                        kT2[64:128, (ck + 1) * KC:(ck + 2) * KC],
                        qT2[64:128, q0 + lo2:q1],
                        start=True, stop=not pdiag, tile_position=(64, 0),
                    )
                    if pdiag:
                        nc.tensor.matmul(
                            ssc[:, 0, lo2:], ident[:],
                            bmask[:, 0:QT - lo2],
                            start=False, stop=True,
                        )
                        nc.tensor.matmul(
                            ssc[:, 1, lo2:], ident[:],
                            bzmask[:, 0:QT - lo2],
                            start=False, stop=True,
                        )
                    p2 = pth2_p.tile([128, 2, QT], BF16, tag="p2", name="p2")
                    nc.scalar.activation(
                        out=p2[:, :, lo2:], in_=ssc[:, :, lo2:],
                        func=EXP, scale=SCALE,
                    )
                    return p2

                # 2-chunk groups. Alloc order ssx(ck)->A, ssx(ck+1)->B,
                # ssc->A keeps every alloc's WAR target two exps old, so
                # each S^T pre-runs inside the previous exp's window and
                # the ACT stream never stalls on the 2-slot psum ring.
                for g in range(nch // 2):
                    ck = 2 * g
                    p01a, loa = st_h01(ck)
                    p01b, lob = st_h01(ck + 1)
                    p2 = st_h2(ck)

                    for h in range(2):
                        nc.tensor.matmul(
                            att[h][:, loa:], vsb[:, ck, h, :],
                            p01a[:, h, loa:],
                            start=(ck == 0), stop=False,
                        )
                        nc.tensor.matmul(
                            att[h][:, lob:], vsb[:, ck + 1, h, :],
                            p01b[:, h, lob:],
                            start=False, stop=(ck + 1 == nch - 1),
                        )
                    for half, cck in enumerate((ck, ck + 1)):
                        jj = cck - 4 * qt
                        hlo = 128 * jj if jj > 0 else 0
                        nc.tensor.matmul(
                            att[2][:, hlo:],
                            vsb[:, cck, 2, :],
                            p2[:, half, hlo:],
                            start=(cck == 0), stop=(cck == nch - 1),
                        )

                    # interleave next block's projection work
                    for _ in range(2):
                        if pending:
                            pending.pop(0)()
                    emit_dummies(2 * DUM_PER_CHUNK, kT01[:, q0:q0 + 128])

                while pending:
                    pending.pop(0)()

                # ---------- epilogue: normalize + c_proj ----------
                ats = []
                for h in range(3):
                    at = at_p.tile([65, QT], BF16, tag="at", name="at")
                    ats.append(at)
                    nc.vector.tensor_copy(at[:], att[h][:])
                    nc.sync.dma_start(out=l_d[qt, h], in_=at[64:65, :])
                linvs = []
                for h in range(3):
                    lcol = lcol_p.tile([128, 4], BF16, tag="lcol", name="lcol")
                    nc.sync.dma_start(
                        out=lcol[:],
                        in_=l_d[qt, h].rearrange("(s p) -> p s", p=128),
                    )
                    linv = lcol_p.tile([128, 4], F32, tag="linv", name="linv")
                    linvs.append(linv)
                    nc.vector.reciprocal(linv[:], lcol[:])

                last = qt == nqt - 1
                emit_dummies(DUM_PER_TILE, kT01[:, q0:q0 + 128])
                for s in range(4):
                    ysb = ysb_p.tile([128, C], F32, tag="ysb", name="ysb")
                    if last:
                        # steal the (now idle) S^T psum ring for parallelism
                        yp_t = ps_p.tile(
                            [128, 2, QT], F32, tag="ps", name="yplast"
                        )
                        yp = yp_t[:, 0, 0:C]
                    else:
                        yp = ydum[:, 0:C]
                    for h in range(3):
                        nc.tensor.matmul(
                            yp,
                            ats[h][0:64, s * 128:(s + 1) * 128],
                            wo[:, h, :],
                            start=True, stop=True,
                        )
                        sc = linvs[h][:, s:s + 1]
                        if h == 0:
                            nc.vector.tensor_scalar(
                                out=ysb[:], in0=yp, scalar1=sc,
                                scalar2=None, op0=MULT,
                            )
                        else:
                            nc.vector.scalar_tensor_tensor(
                                out=ysb[:], in0=yp, scalar=sc, in1=ysb[:],
                                op0=MULT, op1=ADD,
                            )
                    nc.sync.dma_start(
                        out=y_d[q0 + s * 128:q0 + (s + 1) * 128, :],
                        in_=ysb[:],
                    )

    _split_excess_waits(nc)
    nc.finalize()
    return nc


_NC_CACHE = {}


def _get_nc(t=T):
    if t not in _NC_CACHE:
        _NC_CACHE[t] = build(t)
    return _NC_CACHE[t]


def _make_masks(bf16):
    f = np.arange(128)[None, :]
    p = np.arange(128)[:, None]
    tri = (f < p).astype(np.float32) * NEG     # -30000 where q < k
    zero = np.zeros((128, 512), np.float32)
    full = np.full((128, 128), NEG, np.float32)
    bmask = np.concatenate([tri, zero], axis=1)            # [128, 640]
    bzmask = np.concatenate([full, tri, zero], axis=1)     # [128, 768]
    return bmask.astype(bf16), bzmask.astype(bf16)


def _prep_core_inputs(x_b, w_attn, w_proj, hg, bf16):
    """Host-side shard prep for one core: batch x_b, head group hg (0/1)."""
    h0 = 3 * hg
    q = w_attn[:, 0:C]
    k = w_attn[:, C:2 * C]
    v = w_attn[:, 2 * C:3 * C]
    qcols = lambda h: q[:, h * D:(h + 1) * D]
    kcols = lambda h: k[:, h * D:(h + 1) * D]
    w_q01 = np.concatenate([qcols(h0), qcols(h0 + 1)], axis=1)      # [384,128]
    w_k01 = np.concatenate([kcols(h0), kcols(h0 + 1)], axis=1)
    w_q2 = np.concatenate([qcols(h0 + 2), qcols(h0 + 2)], axis=1)   # dup halves
    w_k2 = np.concatenate([kcols(h0 + 2), kcols(h0 + 2)], axis=1)
    w_v = v[:, h0 * D:(h0 + 3) * D]                                 # [384,192]
    w_o = w_proj[h0 * D:(h0 + 3) * D, :]                            # [192,384]
    bmask, bzmask = _make_masks(bf16)
    return {
        "xT16": np.ascontiguousarray(x_b.T, dtype=bf16),
        "w_q01": np.ascontiguousarray(w_q01.reshape(3, 128, 128), dtype=bf16),
        "w_k01": np.ascontiguousarray(w_k01.reshape(3, 128, 128), dtype=bf16),
        "w_q2": np.ascontiguousarray(w_q2.reshape(3, 128, 128), dtype=bf16),
        "w_k2": np.ascontiguousarray(w_k2.reshape(3, 128, 128), dtype=bf16),
        "w_v": np.ascontiguousarray(w_v.reshape(3, 128, 192), dtype=bf16),
        "w_o": np.ascontiguousarray(w_o.reshape(3, 64, 384), dtype=bf16),
        "bmask": bmask,
        "bzmask": bzmask,
    }


def _build_in_maps(x, w_attn, w_proj):
    import ml_dtypes
    bf16 = ml_dtypes.bfloat16
    in_maps = []
    for core in range(8):
        im = _prep_core_inputs(
            x[core // 2], w_attn, w_proj, core % 2, bf16
        )
        in_maps.append(im)
    return in_maps


def kernel(x, w_attn, w_proj):
    x = np.asarray(x, dtype=np.float32)
    w_attn = np.asarray(w_attn, dtype=np.float32)
    w_proj = np.asarray(w_proj, dtype=np.float32)
    b, t, c = x.shape

    nc = _get_nc(t)
    in_maps = _build_in_maps(x, w_attn, w_proj)

    res = run_bass_kernel_spmd(nc, in_maps, list(range(8)))
    out = np.empty((b, t, c), dtype=np.float32)
    for bb in range(b):
        out[bb] = res.results[2 * bb]["y"] + res.results[2 * bb + 1]["y"]
    return out


# revision 17
# speedup vs baseline: 1.5362x; 1.0378x over previous
"""Causal self-attention Trainium2 kernel (B=4, T=4096, C=384, H=6).

Sharding: 8 cores = 4 batches x 2 head-groups (3 heads each). Each core
computes y_partial = attn(x[b], heads hg) @ w_proj[rows of hg]; the host
sums the two partials per batch (the "all-reduce after c_proj" done on
host during unshard).

v3: streaming-ACT design. The exp (ScalarE) stream is the bottleneck
(~220us busy); everything else is structured so ACT never stalls and the
PE never loses its HAM boost clock:
- per-head PSUM accumulators + double-buffered S^T PSUM ring
- small SBUF ring buffers for P^T so cross-tile WAR never stalls exp
- phase-A projections for tile qt+1 interleaved into tile qt's stream
- causal trimming of diagonal chunks (exp/S^T/att@v only on valid cols)
- causal masking applied on the PE as an accumulated bias matmul
  (identity^T @ [-30000*tri | 0...]) so exp flushes masked cols to 0
- lowest-priority "warm keeper" dummy matmuls fill PE idle gaps so the
  HAM clock gate stays at 2.4 GHz
- a fraction of exps computed on the DVE via the Schraudolph int16
  bit-trick to offload the saturated ScalarE
"""

import numpy as np
from contextlib import ExitStack

import concourse.bass as bass
import concourse.tile as tile
from concourse import mybir
from concourse.bass_utils import run_bass_kernel_spmd
from concourse.masks import make_identity
from concourse.vector_clock import ScopedClock

F32 = mybir.dt.float32
BF16 = mybir.dt.bfloat16
I16 = mybir.dt.int16
EXP = mybir.ActivationFunctionType.Exp
MULT = mybir.AluOpType.mult
ADD = mybir.AluOpType.add

B, T, C, H, D = 4, 4096, 384, 6, 64
HPC = 3            # heads per core
QT = 512           # q tile
KC = 128           # key chunk
SCALE = 1.0 / 8.0  # 1/sqrt(64)

NEG = -30000.0     # causal bias; exp(NEG*SCALE) flushes to 0

# Schraudolph exp on DVE (int16/bf16-space): exp(x*SCALE) ~=
# bitcast_bf16(int16(x * SCH_A + SCH_B))
SCHRAUDOLPH = True
SCH_A = (128.0 / float(np.log(2.0))) * SCALE
SCH_B = 16250.5

WARM_DUMMIES = True
DUM_PER_CHUNK = 2
DUM_PER_TILE = 12
DUM_PROLOGUE = 40


# ---------------------------------------------------------------------------
# Workaround: neuronxcc CoreV3 rejects >2 sem waits on the Tile tail drain.
# Split the drain's waits into individual sync-engine wait instructions.
def _drain_and_barrier_split(self, tick_clock, wait_clock):
    nc = self.nc
    drain_inst = nc.sync.drain()
    wait_clock.add_sem_waits(
        drain_inst.ins, ScopedClock({None: tick_clock.global_clock})
    )
    si = drain_inst.ins.sync_info
    if si is not None and si.on_wait and len(si.on_wait) > 1:
        waits = list(si.on_wait)
        si.on_wait = []
        allocated = {h.name: h for h in self.sems.allocated().values()}
        for w in waits:
            h = allocated.get(w.ant_name)
            assert h is not None, f"no sem handle for drain wait {w.ant_name}"
            assert w.wait_mode == "sem-ge-imm", w.wait_mode
            nc.sync.wait_ge(h, w.wait_value)
    nc.all_engine_barrier()
    assert self.sems is not None
    popped = nc._tile_sem_poison_stack.pop()
    assert popped is self._sem_poison
    nc.clear_and_free_semaphores(list(self.sems.allocated().values()))
    nc.all_engine_barrier()


tile.TileContext._drain_and_barrier = _drain_and_barrier_split


MAX_WAITS = 1  # CoreV3 per-instruction sem-wait capacity (S3_LW holds only 1)


def _split_excess_waits(nc):
    """Hoist sem waits beyond MAX_WAITS onto same-engine NOPs inserted
    directly before the over-limit instruction (waits are order-free)."""
    for fn in nc.m.functions:
        for bb in fn.blocks:
            insts = list(bb.instructions)
            out = []
            changed = False
            for inst in insts:
                si = inst.sync_info
                if si is not None and si.on_wait and len(si.on_wait) > MAX_WAITS:
                    waits = list(si.on_wait)
                    excess, keep = waits[:-MAX_WAITS], waits[-MAX_WAITS:]
                    si.on_wait = keep
                    inst.sync_info = si
                    for i in range(0, len(excess), MAX_WAITS):
                        nop = mybir.InstNoOp(
                            name=f"{inst.name}-waitsplit-{i}", ins=[], outs=[]
                        )
                        nop.engine = inst.engine
                        nop.sync_info = mybir.SyncInfo(
                            on_wait=excess[i:i + MAX_WAITS], on_update=[]
                        )
                        nc.register_instruction(nop)
                        out.append(nop)
                    changed = True
                out.append(inst)
            if changed:
                bb.instructions = out
# ---------------------------------------------------------------------------


def build(t=T):
    nqt = t // QT          # q tiles
    nkc = t // KC          # key chunks

    nc = bass.Bass()
    x_d = nc.dram_tensor("xT16", [C, t], BF16, kind="ExternalInput")
    wq01_d = nc.dram_tensor("w_q01", [3, 128, 128], BF16, kind="ExternalInput")
    wk01_d = nc.dram_tensor("w_k01", [3, 128, 128], BF16, kind="ExternalInput")
    # head-2 q/k weights duplicated into both column halves so the
    # projection fills both partition halves directly
    wq2_d = nc.dram_tensor("w_q2", [3, 128, 128], BF16, kind="ExternalInput")
    wk2_d = nc.dram_tensor("w_k2", [3, 128, 128], BF16, kind="ExternalInput")
    wv_d = nc.dram_tensor("w_v", [3, 128, 192], BF16, kind="ExternalInput")
    wo_d = nc.dram_tensor("w_o", [3, 64, 384], BF16, kind="ExternalInput")
    bmask_d = nc.dram_tensor("bmask", [128, 640], BF16, kind="ExternalInput")
    bzmask_d = nc.dram_tensor("bzmask", [128, 768], BF16, kind="ExternalInput")
    y_d = nc.dram_tensor("y", [t, C], F32, kind="ExternalOutput")
    # scratch for transposing the softmax denominator row into columns
    l_d = nc.dram_tensor("lscratch", [t // QT, 3, QT], BF16)

    with tile.TileContext(nc) as tc, ExitStack() as ctx:
        persist = ctx.enter_context(tc.tile_pool(name="persist", bufs=1))

        # weights / masks
        wq01 = persist.tile([128, 3, 128], BF16)
        wk01 = persist.tile([128, 3, 128], BF16)
        wq2 = persist.tile([128, 3, 128], BF16)
        wk2 = persist.tile([128, 3, 128], BF16)
        wv = persist.tile([128, 3, 192], BF16)
        wo = persist.tile([64, 3, 384], BF16)
        for c in range(3):
            nc.sync.dma_start(out=wq01[:, c, :], in_=wq01_d[c])
            nc.sync.dma_start(out=wk01[:, c, :], in_=wk01_d[c])
            nc.sync.dma_start(out=wq2[:, c, :], in_=wq2_d[c])
            nc.sync.dma_start(out=wk2[:, c, :], in_=wk2_d[c])
            nc.sync.dma_start(out=wv[:, c, :], in_=wv_d[c])
            nc.sync.dma_start(out=wo[:, c, :], in_=wo_d[c])
        bmask = persist.tile([128, 640], BF16)
        bzmask = persist.tile([128, 768], BF16)
        nc.sync.dma_start(out=bmask[:], in_=bmask_d[:])
        nc.sync.dma_start(out=bzmask[:], in_=bzmask_d[:])
        ident = persist.tile([128, 128], BF16)
        make_identity(nc, ident[:])

        # persistent activations (bf16)
        qT01 = persist.tile([128, t], BF16)   # rows 0:64 h0 qT, 64:128 h1 qT
        kT01 = persist.tile([128, t], BF16)
        qT2 = persist.tile([128, t], BF16)    # head 2 duplicated both halves
        kT2 = persist.tile([128, t], BF16)
        vsb = persist.tile([128, nkc, 3, 65], BF16)  # [keys, chunk, head, d|one]
        nc.vector.memset(vsb[:, :, :, 64:65], 1.0)

        with (
            tc.tile_pool(name="xt", bufs=2) as xt_p,
            tc.tile_pool(name="ps", bufs=2, space="PSUM") as ps_p,
            tc.tile_pool(name="attps", bufs=1, space="PSUM") as att_p,
            tc.tile_pool(name="yps", bufs=1, space="PSUM") as y_p,
            tc.tile_pool(name="pth01", bufs=6) as pth01_p,
            tc.tile_pool(name="pthi", bufs=3) as pthi_p,
            tc.tile_pool(name="pth2", bufs=4) as pth2_p,
            tc.tile_pool(name="atsb", bufs=6) as at_p,
            tc.tile_pool(name="lcol", bufs=6) as lcol_p,
            tc.tile_pool(name="yout", bufs=3) as ysb_p,
        ):
            # one PSUM bank shared by c_proj output (cols 0:384) and the
            # warm-keeper dummy target (cols 384:512)
            ydum = y_p.tile([128, QT], F32, tag="y", name="ydum")

            def emit_dummies(n, rhs):
                if not WARM_DUMMIES:
                    return
                with tc.high_priority(offset=-(10 ** 9)):
                    for _ in range(n):
                        nc.tensor.matmul(
                            ydum[:, 384:512], wq01[:, 0, :], rhs,
                            start=True, stop=True,
                        )

            # ---------- phase A: projections for one 512-token block ------
            def phase_a_ops(tb):
                """Emit the x DMA immediately; return closures for the
                projection groups: 2 consolidated q/k pair groups on the
                S^T psum ring + 4 v sub-blocks accumulated in the ydum
                bank (zero ring pressure)."""
                xT = xt_p.tile([128, 3, QT], BF16, tag="xt", name="xT")
                for c in range(3):
                    nc.sync.dma_start(
                        out=xT[:, c, :],
                        in_=x_d[c * 128:(c + 1) * 128, tb * QT:(tb + 1) * QT],
                    )
                ops = []

                def qk_group(w_sb, dst):
                    def op():
                        ps = ps_p.tile([128, 2, QT], F32, tag="ps", name="psqk")
                        for c in range(3):
                            nc.tensor.matmul(
                                ps[:, 0, :], w_sb[:, c, :], xT[:, c, :],
                                start=(c == 0), stop=(c == 2),
                            )
                        nc.vector.tensor_copy(
                            dst[:, tb * QT:(tb + 1) * QT], ps[:, 0, :]
                        )
                    return op

                def v_group(s):
                    def op():
                        for c in range(3):
                            nc.tensor.matmul(
                                ydum[:, 0:192],
                                xT[:, c, s * 128:(s + 1) * 128],
                                wv[:, c, :],
                                start=(c == 0), stop=(c == 2),
                            )
                        nc.vector.tensor_copy(
                            vsb[:, tb * 4 + s, :, 0:64],
                            ydum[:, 0:192].rearrange(
                                "p (h d) -> p h d", h=3
                            ),
                        )
                    return op

                ops.append(qk_group(wq01, qT01))
                ops.append(qk_group(wk01, kT01))
                ops.append(qk_group(wq2, qT2))
                ops.append(qk_group(wk2, kT2))
                for s in range(4):
                    ops.append(v_group(s))
                return ops

            # prologue: warm the PE, then block-0 projections
            emit_dummies(DUM_PROLOGUE, wk01[:, 0, :])
            for op in phase_a_ops(0):
                op()

            sch_count = 0
            for qt in range(nqt):
                nch = 4 * (qt + 1)
                q0, q1 = qt * QT, (qt + 1) * QT
                pending = phase_a_ops(qt + 1) if qt + 1 < nqt else []

                att = [
                    att_p.tile([65, QT], F32, tag=f"att{h}", name=f"att{h}")
                    for h in range(3)
                ]

                def st_h01(ck):
                    """S^T + exp (or Schraudolph) for heads 0/1, one chunk.
                    Returns the bf16 P^T view [128, 2, QT]."""
                    j = ck - 4 * qt
                    diag = j >= 0
                    lo = 128 * j if j > 0 else 0
                    use_sch = SCHRAUDOLPH and not diag and ck % 3 == 1
                    ssx = ps_p.tile([128, 2, QT], F32, tag="ps", name="ssx")
                    with tc.high_priority():
                        nc.tensor.matmul(
                            ssx[:, 0, lo:],
                            kT01[0:64, ck * KC:(ck + 1) * KC],
                            qT01[0:64, q0 + lo:q1],
                            start=True, stop=not diag, tile_position=(0, 0),
                        )
                        nc.tensor.matmul(
                            ssx[:, 1, lo:],
                            kT01[64:128, ck * KC:(ck + 1) * KC],
                            qT01[64:128, q0 + lo:q1],
                            start=True, stop=not diag, tile_position=(64, 0),
                        )
                        if diag:
                            # causal bias: += ident^T @ [-30000*tri | zeros]
                            for h in range(2):
                                nc.tensor.matmul(
                                    ssx[:, h, lo:], ident[:],
                                    bmask[:, 0:QT - lo],
                                    start=False, stop=True,
                                )
                    if use_sch:
                        p01i = pthi_p.tile(
                            [128, 2, QT], I16, tag="p01i", name="p01i"
                        )
                        nc.vector.tensor_scalar(
                            out=p01i[:, :, :], in0=ssx[:, :, :],
                            scalar1=SCH_A, scalar2=SCH_B,
                            op0=MULT, op1=ADD,
                        )
                        return p01i.bitcast(BF16), lo
                    p01 = pth01_p.tile(
                        [128, 2, QT], BF16, tag="p01", name="p01"
                    )
                    nc.scalar.activation(
                        out=p01[:, :, lo:], in_=ssx[:, :, lo:],
                        func=EXP, scale=SCALE,
                    )
                    return p01, lo

                def st_h2(ck):
                    """S^T + exp for head 2, chunk pair (ck, ck+1) packed in
                    concurrent row groups."""
                    jp = ck - 4 * qt
                    pdiag = jp >= 0
                    lo2 = 128 * jp if jp > 0 else 0
                    ssc = ps_p.tile([128, 2, QT], F32, tag="ps", name="ssc")
                    with tc.high_priority():
                        nc.tensor.matmul(
                            ssc[:, 0, lo2:],
                            kT2[0:64, ck * KC:(ck + 1) * KC],
                            qT2[0:64, q0 + lo2:q1],
                            start=True, stop=not pdiag, tile_position=(0, 0),
                        )
                        nc.tensor.matmul(
                            ssc[:, 1, lo2:],
                            kT2[64:128, (ck + 1) * KC:(ck + 2) * KC],
                            qT2[64:128, q0 + lo2:q1],
                            start=True, stop=not pdiag, tile_position=(64, 0),
                        )
                        if pdiag:
                            nc.tensor.matmul(
                                ssc[:, 0, lo2:], ident[:],
                                bmask[:, 0:QT - lo2],
                                start=False, stop=True,
                            )
                            nc.tensor.matmul(
                                ssc[:, 1, lo2:], ident[:],
                                bzmask[:, 0:QT - lo2],
                                start=False, stop=True,
                            )
                    p2 = pth2_p.tile([128, 2, QT], BF16, tag="p2", name="p2")
                    nc.scalar.activation(
                        out=p2[:, :, lo2:], in_=ssc[:, :, lo2:],
                        func=EXP, scale=SCALE,
                    )
                    return p2

                # 2-chunk groups. Alloc order ssx(ck)->A, ssx(ck+1)->B,
                # ssc->A keeps every alloc's WAR target two exps old, so
                # each S^T pre-runs inside the previous exp's window and
                # the ACT stream never stalls on the 2-slot psum ring.
                for g in range(nch // 2):
                    ck = 2 * g
                    p01a, loa = st_h01(ck)
                    p01b, lob = st_h01(ck + 1)
                    p2 = st_h2(ck)

                    for h in range(2):
                        nc.tensor.matmul(
                            att[h][:, loa:], vsb[:, ck, h, :],
                            p01a[:, h, loa:],
                            start=(ck == 0), stop=False,
                        )
                        nc.tensor.matmul(
                            att[h][:, lob:], vsb[:, ck + 1, h, :],
                            p01b[:, h, lob:],
                            start=False, stop=(ck + 1 == nch - 1),
                        )
                    for half, cck in enumerate((ck, ck + 1)):
                        jj = cck - 4 * qt
                        hlo = 128 * jj if jj > 0 else 0
                        nc.tensor.matmul(
                            att[2][:, hlo:],
                            vsb[:, cck, 2, :],
                            p2[:, half, hlo:],
                            start=(cck == 0), stop=(cck == nch - 1),
                        )

                    # interleave next block's projection work, spread so
                    # nothing drains in a clump at the tile boundary
                    groups_left = nch // 2 - g
                    npop = -(-len(pending) // groups_left)  # ceil
                    for _ in range(npop):
                        if pending:
                            pending.pop(0)()
                    emit_dummies(2 * DUM_PER_CHUNK, kT01[:, q0:q0 + 128])

                while pending:
                    pending.pop(0)()

                # ---------- epilogue: normalize + c_proj ----------
                ats = []
                for h in range(3):
                    at = at_p.tile([65, QT], BF16, tag="at", name="at")
                    ats.append(at)
                    nc.vector.tensor_copy(at[:], att[h][:])
                    nc.sync.dma_start(out=l_d[qt, h], in_=at[64:65, :])
                linvs = []
                for h in range(3):
                    lcol = lcol_p.tile([128, 4], BF16, tag="lcol", name="lcol")
                    nc.sync.dma_start(
                        out=lcol[:],
                        in_=l_d[qt, h].rearrange("(s p) -> p s", p=128),
                    )
                    linv = lcol_p.tile([128, 4], F32, tag="linv", name="linv")
                    linvs.append(linv)
                    nc.vector.reciprocal(linv[:], lcol[:])

                last = qt == nqt - 1
                emit_dummies(DUM_PER_TILE, kT01[:, q0:q0 + 128])
                for s in range(4):
                    ysb = ysb_p.tile([128, C], F32, tag="ysb", name="ysb")
                    if last:
                        # steal the (now idle) S^T psum ring for parallelism
                        yp_t = ps_p.tile(
                            [128, 2, QT], F32, tag="ps", name="yplast"
                        )
                        yp = yp_t[:, 0, 0:C]
                    else:
                        yp = ydum[:, 0:C]
                    for h in range(3):
                        nc.tensor.matmul(
                            yp,
                            ats[h][0:64, s * 128:(s + 1) * 128],
                            wo[:, h, :],
                            start=True, stop=True,
                        )
                        sc = linvs[h][:, s:s + 1]
                        if h == 0:
                            nc.vector.tensor_scalar(
                                out=ysb[:], in0=yp, scalar1=sc,
                                scalar2=None, op0=MULT,
                            )
                        else:
                            nc.vector.scalar_tensor_tensor(
                                out=ysb[:], in0=yp, scalar=sc, in1=ysb[:],
                                op0=MULT, op1=ADD,
                            )
                    nc.sync.dma_start(
                        out=y_d[q0 + s * 128:q0 + (s + 1) * 128, :],
                        in_=ysb[:],
                    )

    _split_excess_waits(nc)
    nc.finalize()
    return nc


_NC_CACHE = {}


def _get_nc(t=T):
    if t not in _NC_CACHE:
        _NC_CACHE[t] = build(t)
    return _NC_CACHE[t]


def _make_masks(bf16):
    f = np.arange(128)[None, :]
    p = np.arange(128)[:, None]
    tri = (f < p).astype(np.float32) * NEG     # -30000 where q < k
    zero = np.zeros((128, 512), np.float32)
    full = np.full((128, 128), NEG, np.float32)
    bmask = np.concatenate([tri, zero], axis=1)            # [128, 640]
    bzmask = np.concatenate([full, tri, zero], axis=1)     # [128, 768]
    return bmask.astype(bf16), bzmask.astype(bf16)


def _prep_core_inputs(x_b, w_attn, w_proj, hg, bf16):
    """Host-side shard prep for one core: batch x_b, head group hg (0/1)."""
    h0 = 3 * hg
    q = w_attn[:, 0:C]
    k = w_attn[:, C:2 * C]
    v = w_attn[:, 2 * C:3 * C]
    qcols = lambda h: q[:, h * D:(h + 1) * D]
    kcols = lambda h: k[:, h * D:(h + 1) * D]
    w_q01 = np.concatenate([qcols(h0), qcols(h0 + 1)], axis=1)      # [384,128]
    w_k01 = np.concatenate([kcols(h0), kcols(h0 + 1)], axis=1)
    w_q2 = np.concatenate([qcols(h0 + 2), qcols(h0 + 2)], axis=1)   # dup halves
    w_k2 = np.concatenate([kcols(h0 + 2), kcols(h0 + 2)], axis=1)
    w_v = v[:, h0 * D:(h0 + 3) * D]                                 # [384,192]
    w_o = w_proj[h0 * D:(h0 + 3) * D, :]                            # [192,384]
    bmask, bzmask = _make_masks(bf16)
    return {
        "xT16": np.ascontiguousarray(x_b.T, dtype=bf16),
        "w_q01": np.ascontiguousarray(w_q01.reshape(3, 128, 128), dtype=bf16),
        "w_k01": np.ascontiguousarray(w_k01.reshape(3, 128, 128), dtype=bf16),
        "w_q2": np.ascontiguousarray(w_q2.reshape(3, 128, 128), dtype=bf16),
        "w_k2": np.ascontiguousarray(w_k2.reshape(3, 128, 128), dtype=bf16),
        "w_v": np.ascontiguousarray(w_v.reshape(3, 128, 192), dtype=bf16),
        "w_o": np.ascontiguousarray(w_o.reshape(3, 64, 384), dtype=bf16),
        "bmask": bmask,
        "bzmask": bzmask,
    }


def _build_in_maps(x, w_attn, w_proj):
    import ml_dtypes
    bf16 = ml_dtypes.bfloat16
    in_maps = []
    for core in range(8):
        im = _prep_core_inputs(
            x[core // 2], w_attn, w_proj, core % 2, bf16
        )
        in_maps.append(im)
    return in_maps


def kernel(x, w_attn, w_proj):
    x = np.asarray(x, dtype=np.float32)
    w_attn = np.asarray(w_attn, dtype=np.float32)
    w_proj = np.asarray(w_proj, dtype=np.float32)
    b, t, c = x.shape

    nc = _get_nc(t)
    in_maps = _build_in_maps(x, w_attn, w_proj)

    res = run_bass_kernel_spmd(nc, in_maps, list(range(8)))
    out = np.empty((b, t, c), dtype=np.float32)
    for bb in range(b):
        out[bb] = res.results[2 * bb]["y"] + res.results[2 * bb + 1]["y"]
    return out


# revision 19
# speedup vs baseline: 1.5403x; 1.0027x over previous
"""Causal self-attention Trainium2 kernel (B=4, T=4096, C=384, H=6).

Sharding: 8 cores = 4 batches x 2 head-groups (3 heads each). Each core
computes y_partial = attn(x[b], heads hg) @ w_proj[rows of hg]; the host
sums the two partials per batch (the "all-reduce after c_proj" done on
host during unshard).

v3: streaming-ACT design. The exp (ScalarE) stream is the bottleneck
(~220us busy); everything else is structured so ACT never stalls and the
PE never loses its HAM boost clock:
- per-head PSUM accumulators + double-buffered S^T PSUM ring
- small SBUF ring buffers for P^T so cross-tile WAR never stalls exp
- phase-A projections for tile qt+1 interleaved into tile qt's stream
- causal trimming of diagonal chunks (exp/S^T/att@v only on valid cols)
- causal masking applied on the PE as an accumulated bias matmul
  (identity^T @ [-30000*tri | 0...]) so exp flushes masked cols to 0
- lowest-priority "warm keeper" dummy matmuls fill PE idle gaps so the
  HAM clock gate stays at 2.4 GHz
- a fraction of exps computed on the DVE via the Schraudolph int16
  bit-trick to offload the saturated ScalarE
"""

import numpy as np
from contextlib import ExitStack

import concourse.bass as bass
import concourse.tile as tile
from concourse import mybir
from concourse.bass_utils import run_bass_kernel_spmd
from concourse.masks import make_identity
from concourse.vector_clock import ScopedClock

F32 = mybir.dt.float32
BF16 = mybir.dt.bfloat16
I16 = mybir.dt.int16
EXP = mybir.ActivationFunctionType.Exp
MULT = mybir.AluOpType.mult
ADD = mybir.AluOpType.add

B, T, C, H, D = 4, 4096, 384, 6, 64
HPC = 3            # heads per core
QT = 512           # q tile
KC = 128           # key chunk
SCALE = 1.0 / 8.0  # 1/sqrt(64)

NEG = -30000.0     # causal bias; exp(NEG*SCALE) flushes to 0

# Schraudolph exp on DVE (int16/bf16-space): exp(x*SCALE) ~=
# bitcast_bf16(int16(x * SCH_A + SCH_B))
SCHRAUDOLPH = True
SCH_A = (128.0 / float(np.log(2.0))) * SCALE
SCH_B = 16250.5

WARM_DUMMIES = True
DUM_PER_CHUNK = 2
DUM_PER_TILE = 12
DUM_PROLOGUE = 40


# ---------------------------------------------------------------------------
# Workaround: neuronxcc CoreV3 rejects >2 sem waits on the Tile tail drain.
# Split the drain's waits into individual sync-engine wait instructions.
def _drain_and_barrier_split(self, tick_clock, wait_clock):
    nc = self.nc
    drain_inst = nc.sync.drain()
    wait_clock.add_sem_waits(
        drain_inst.ins, ScopedClock({None: tick_clock.global_clock})
    )
    si = drain_inst.ins.sync_info
    if si is not None and si.on_wait and len(si.on_wait) > 1:
        waits = list(si.on_wait)
        si.on_wait = []
        allocated = {h.name: h for h in self.sems.allocated().values()}
        for w in waits:
            h = allocated.get(w.ant_name)
            assert h is not None, f"no sem handle for drain wait {w.ant_name}"
            assert w.wait_mode == "sem-ge-imm", w.wait_mode
            nc.sync.wait_ge(h, w.wait_value)
    nc.all_engine_barrier()
    assert self.sems is not None
    popped = nc._tile_sem_poison_stack.pop()
    assert popped is self._sem_poison
    nc.clear_and_free_semaphores(list(self.sems.allocated().values()))
    nc.all_engine_barrier()


tile.TileContext._drain_and_barrier = _drain_and_barrier_split


MAX_WAITS = 1  # CoreV3 per-instruction sem-wait capacity (S3_LW holds only 1)


def _split_excess_waits(nc):
    """Hoist sem waits beyond MAX_WAITS onto same-engine NOPs inserted
    directly before the over-limit instruction (waits are order-free)."""
    for fn in nc.m.functions:
        for bb in fn.blocks:
            insts = list(bb.instructions)
            out = []
            changed = False
            for inst in insts:
                si = inst.sync_info
                if si is not None and si.on_wait and len(si.on_wait) > MAX_WAITS:
                    waits = list(si.on_wait)
                    excess, keep = waits[:-MAX_WAITS], waits[-MAX_WAITS:]
                    si.on_wait = keep
                    inst.sync_info = si
                    for i in range(0, len(excess), MAX_WAITS):
                        nop = mybir.InstNoOp(
                            name=f"{inst.name}-waitsplit-{i}", ins=[], outs=[]
                        )
                        nop.engine = inst.engine
                        nop.sync_info = mybir.SyncInfo(
                            on_wait=excess[i:i + MAX_WAITS], on_update=[]
                        )
                        nc.register_instruction(nop)
                        out.append(nop)
                    changed = True
                out.append(inst)
            if changed:
                bb.instructions = out
# ---------------------------------------------------------------------------


def build(t=T):
    nqt = t // QT          # q tiles
    nkc = t // KC          # key chunks

    nc = bass.Bass()
    x_d = nc.dram_tensor("xT16", [C, t], BF16, kind="ExternalInput")
    wq01_d = nc.dram_tensor("w_q01", [3, 128, 128], BF16, kind="ExternalInput")
    wk01_d = nc.dram_tensor("w_k01", [3, 128, 128], BF16, kind="ExternalInput")
    # head-2 q/k weights duplicated into both column halves so the
    # projection fills both partition halves directly
    wq2_d = nc.dram_tensor("w_q2", [3, 128, 128], BF16, kind="ExternalInput")
    wk2_d = nc.dram_tensor("w_k2", [3, 128, 128], BF16, kind="ExternalInput")
    wv_d = nc.dram_tensor("w_v", [3, 128, 192], BF16, kind="ExternalInput")
    wo_d = nc.dram_tensor("w_o", [3, 64, 384], BF16, kind="ExternalInput")
    bmask_d = nc.dram_tensor("bmask", [128, 640], BF16, kind="ExternalInput")
    bzmask_d = nc.dram_tensor("bzmask", [128, 768], BF16, kind="ExternalInput")
    y_d = nc.dram_tensor("y", [t, C], F32, kind="ExternalOutput")
    # scratch for transposing the softmax denominator row into columns
    l_d = nc.dram_tensor("lscratch", [t // QT, 3, QT], BF16)

    with tile.TileContext(nc) as tc, ExitStack() as ctx:
        persist = ctx.enter_context(tc.tile_pool(name="persist", bufs=1))

        # weights / masks
        wq01 = persist.tile([128, 3, 128], BF16)
        wk01 = persist.tile([128, 3, 128], BF16)
        wq2 = persist.tile([128, 3, 128], BF16)
        wk2 = persist.tile([128, 3, 128], BF16)
        wv = persist.tile([128, 3, 192], BF16)
        wo = persist.tile([64, 3, 384], BF16)
        for c in range(3):
            nc.sync.dma_start(out=wq01[:, c, :], in_=wq01_d[c])
            nc.sync.dma_start(out=wk01[:, c, :], in_=wk01_d[c])
            nc.sync.dma_start(out=wq2[:, c, :], in_=wq2_d[c])
            nc.sync.dma_start(out=wk2[:, c, :], in_=wk2_d[c])
            nc.sync.dma_start(out=wv[:, c, :], in_=wv_d[c])
            nc.sync.dma_start(out=wo[:, c, :], in_=wo_d[c])
        bmask = persist.tile([128, 640], BF16)
        bzmask = persist.tile([128, 768], BF16)
        nc.sync.dma_start(out=bmask[:], in_=bmask_d[:])
        nc.sync.dma_start(out=bzmask[:], in_=bzmask_d[:])
        ident = persist.tile([128, 128], BF16)
        make_identity(nc, ident[:])

        # persistent activations (bf16)
        qT01 = persist.tile([128, t], BF16)   # rows 0:64 h0 qT, 64:128 h1 qT
        kT01 = persist.tile([128, t], BF16)
        qT2 = persist.tile([128, t], BF16)    # head 2 duplicated both halves
        kT2 = persist.tile([128, t], BF16)
        vsb = persist.tile([128, nkc, 3, 65], BF16)  # [keys, chunk, head, d|one]
        nc.vector.memset(vsb[:, :, :, 64:65], 1.0)

        with (
            tc.tile_pool(name="xt", bufs=2) as xt_p,
            tc.tile_pool(name="ps", bufs=2, space="PSUM") as ps_p,
            tc.tile_pool(name="attps", bufs=1, space="PSUM") as att_p,
            tc.tile_pool(name="yps", bufs=1, space="PSUM") as y_p,
            tc.tile_pool(name="pth01", bufs=6) as pth01_p,
            tc.tile_pool(name="pthi", bufs=3) as pthi_p,
            tc.tile_pool(name="pth2", bufs=4) as pth2_p,
            tc.tile_pool(name="atsb", bufs=6) as at_p,
            tc.tile_pool(name="lcol", bufs=6) as lcol_p,
            tc.tile_pool(name="yout", bufs=3) as ysb_p,
        ):
            # one PSUM bank shared by c_proj output (cols 0:384) and the
            # warm-keeper dummy target (cols 384:512)
            ydum = y_p.tile([128, QT], F32, tag="y", name="ydum")

            def emit_dummies(n, rhs):
                if not WARM_DUMMIES:
                    return
                with tc.high_priority(offset=-(10 ** 9)):
                    for _ in range(n):
                        nc.tensor.matmul(
                            ydum[:, 384:512], wq01[:, 0, :], rhs,
                            start=True, stop=True,
                        )

            # ---------- phase A: projections for one 512-token block ------
            def phase_a_ops(tb):
                """Emit the x DMA immediately; return closures for the
                projection groups: 2 consolidated q/k pair groups on the
                S^T psum ring + 4 v sub-blocks accumulated in the ydum
                bank (zero ring pressure)."""
                xT = xt_p.tile([128, 3, QT], BF16, tag="xt", name="xT")
                for c in range(3):
                    nc.sync.dma_start(
                        out=xT[:, c, :],
                        in_=x_d[c * 128:(c + 1) * 128, tb * QT:(tb + 1) * QT],
                    )
                ops = []

                def qk_group(w_sb, dst):
                    def op():
                        ps = ps_p.tile([128, 2, QT], F32, tag="ps", name="psqk")
                        for c in range(3):
                            nc.tensor.matmul(
                                ps[:, 0, :], w_sb[:, c, :], xT[:, c, :],
                                start=(c == 0), stop=(c == 2),
                            )
                        nc.vector.tensor_copy(
                            dst[:, tb * QT:(tb + 1) * QT], ps[:, 0, :]
                        )
                    return op

                def v_group(s):
                    def op():
                        for c in range(3):
                            nc.tensor.matmul(
                                ydum[:, 0:192],
                                xT[:, c, s * 128:(s + 1) * 128],
                                wv[:, c, :],
                                start=(c == 0), stop=(c == 2),
                            )
                        nc.vector.tensor_copy(
                            vsb[:, tb * 4 + s, :, 0:64],
                            ydum[:, 0:192].rearrange(
                                "p (h d) -> p h d", h=3
                            ),
                        )
                    return op

                ops.append(qk_group(wq01, qT01))
                ops.append(qk_group(wk01, kT01))
                ops.append(qk_group(wq2, qT2))
                ops.append(qk_group(wk2, kT2))
                for s in range(4):
                    ops.append(v_group(s))
                return ops

            # prologue: warm the PE, then block-0 projections
            emit_dummies(DUM_PROLOGUE, wk01[:, 0, :])
            for op in phase_a_ops(0):
                op()

            sch_count = 0
            for qt in range(nqt):
                nch = 4 * (qt + 1)
                q0, q1 = qt * QT, (qt + 1) * QT
                pending = phase_a_ops(qt + 1) if qt + 1 < nqt else []

                att = [
                    att_p.tile([65, QT], F32, tag=f"att{h}", name=f"att{h}")
                    for h in range(3)
                ]

                def st_h01(ck):
                    """S^T + exp (or Schraudolph) for heads 0/1, one chunk.
                    Returns the bf16 P^T view [128, 2, QT]."""
                    j = ck - 4 * qt
                    diag = j >= 0
                    lo = 128 * j if j > 0 else 0
                    use_sch = SCHRAUDOLPH and not diag and ck % 3 == 1
                    ssx = ps_p.tile([128, 2, QT], F32, tag="ps", name="ssx")
                    with tc.high_priority():
                        nc.tensor.matmul(
                            ssx[:, 0, lo:],
                            kT01[0:64, ck * KC:(ck + 1) * KC],
                            qT01[0:64, q0 + lo:q1],
                            start=True, stop=not diag, tile_position=(0, 0),
                        )
                        nc.tensor.matmul(
                            ssx[:, 1, lo:],
                            kT01[64:128, ck * KC:(ck + 1) * KC],
                            qT01[64:128, q0 + lo:q1],
                            start=True, stop=not diag, tile_position=(64, 0),
                        )
                        if diag:
                            # causal bias: += ident^T @ [-30000*tri | zeros]
                            for h in range(2):
                                nc.tensor.matmul(
                                    ssx[:, h, lo:], ident[:],
                                    bmask[:, 0:QT - lo],
                                    start=False, stop=True,
                                )
                    if use_sch:
                        p01i = pthi_p.tile(
                            [128, 2, QT], I16, tag="p01i", name="p01i"
                        )
                        with tc.high_priority():
                            nc.vector.tensor_scalar(
                                out=p01i[:, :, :], in0=ssx[:, :, :],
                                scalar1=SCH_A, scalar2=SCH_B,
                                op0=MULT, op1=ADD,
                            )
                        return p01i.bitcast(BF16), lo
                    p01 = pth01_p.tile(
                        [128, 2, QT], BF16, tag="p01", name="p01"
                    )
                    nc.scalar.activation(
                        out=p01[:, :, lo:], in_=ssx[:, :, lo:],
                        func=EXP, scale=SCALE,
                    )
                    return p01, lo

                def st_h2(ck):
                    """S^T + exp for head 2, chunk pair (ck, ck+1) packed in
                    concurrent row groups."""
                    jp = ck - 4 * qt
                    pdiag = jp >= 0
                    lo2 = 128 * jp if jp > 0 else 0
                    ssc = ps_p.tile([128, 2, QT], F32, tag="ps", name="ssc")
                    with tc.high_priority():
                        nc.tensor.matmul(
                            ssc[:, 0, lo2:],
                            kT2[0:64, ck * KC:(ck + 1) * KC],
                            qT2[0:64, q0 + lo2:q1],
                            start=True, stop=not pdiag, tile_position=(0, 0),
                        )
                        nc.tensor.matmul(
                            ssc[:, 1, lo2:],
                            kT2[64:128, (ck + 1) * KC:(ck + 2) * KC],
                            qT2[64:128, q0 + lo2:q1],
                            start=True, stop=not pdiag, tile_position=(64, 0),
                        )
                        if pdiag:
                            nc.tensor.matmul(
                                ssc[:, 0, lo2:], ident[:],
                                bmask[:, 0:QT - lo2],
                                start=False, stop=True,
                            )
                            nc.tensor.matmul(
                                ssc[:, 1, lo2:], ident[:],
                                bzmask[:, 0:QT - lo2],
                                start=False, stop=True,
                            )
                    p2 = pth2_p.tile([128, 2, QT], BF16, tag="p2", name="p2")
                    nc.scalar.activation(
                        out=p2[:, :, lo2:], in_=ssc[:, :, lo2:],
                        func=EXP, scale=SCALE,
                    )
                    return p2

                # 2-chunk groups. Alloc order ssx(ck)->A, ssx(ck+1)->B,
                # ssc->A keeps every alloc's WAR target two exps old, so
                # each S^T pre-runs inside the previous exp's window and
                # the ACT stream never stalls on the 2-slot psum ring.
                for g in range(nch // 2):
                    ck = 2 * g
                    p01a, loa = st_h01(ck)
                    p01b, lob = st_h01(ck + 1)
                    p2 = st_h2(ck)

                    for h in range(2):
                        nc.tensor.matmul(
                            att[h][:, loa:], vsb[:, ck, h, :],
                            p01a[:, h, loa:],
                            start=(ck == 0), stop=False,
                        )
                        nc.tensor.matmul(
                            att[h][:, lob:], vsb[:, ck + 1, h, :],
                            p01b[:, h, lob:],
                            start=False, stop=(ck + 1 == nch - 1),
                        )
                    for half, cck in enumerate((ck, ck + 1)):
                        jj = cck - 4 * qt
                        hlo = 128 * jj if jj > 0 else 0
                        nc.tensor.matmul(
                            att[2][:, hlo:],
                            vsb[:, cck, 2, :],
                            p2[:, half, hlo:],
                            start=(cck == 0), stop=(cck == nch - 1),
                        )

                    # interleave next block's projection work, spread so
                    # nothing drains in a clump at the tile boundary
                    groups_left = nch // 2 - g
                    npop = min(2, -(-len(pending) // groups_left))  # ceil
                    for _ in range(npop):
                        if pending:
                            pending.pop(0)()
                    emit_dummies(2 * DUM_PER_CHUNK, kT01[:, q0:q0 + 128])

                while pending:
                    pending.pop(0)()

                # ---------- epilogue: normalize + c_proj ----------
                ats = []
                for h in range(3):
                    at = at_p.tile([65, QT], BF16, tag="at", name="at")
                    ats.append(at)
                    nc.vector.tensor_copy(at[:], att[h][:])
                    nc.sync.dma_start(out=l_d[qt, h], in_=at[64:65, :])
                linvs = []
                for h in range(3):
                    lcol = lcol_p.tile([128, 4], BF16, tag="lcol", name="lcol")
                    nc.sync.dma_start(
                        out=lcol[:],
                        in_=l_d[qt, h].rearrange("(s p) -> p s", p=128),
                    )
                    linv = lcol_p.tile([128, 4], F32, tag="linv", name="linv")
                    linvs.append(linv)
                    nc.vector.reciprocal(linv[:], lcol[:])

                last = qt == nqt - 1
                emit_dummies(DUM_PER_TILE, kT01[:, q0:q0 + 128])
                for s in range(4):
                    ysb = ysb_p.tile([128, C], F32, tag="ysb", name="ysb")
                    if last:
                        # steal the (now idle) S^T psum ring for parallelism
                        yp_t = ps_p.tile(
                            [128, 2, QT], F32, tag="ps", name="yplast"
                        )
                        yp = yp_t[:, 0, 0:C]
                    else:
                        yp = ydum[:, 0:C]
                    for h in range(3):
                        nc.tensor.matmul(
                            yp,
                            ats[h][0:64, s * 128:(s + 1) * 128],
                            wo[:, h, :],
                            start=True, stop=True,
                        )
                        sc = linvs[h][:, s:s + 1]
                        if h == 0:
                            nc.vector.tensor_scalar(
                                out=ysb[:], in0=yp, scalar1=sc,
                                scalar2=None, op0=MULT,
                            )
                        else:
                            nc.vector.scalar_tensor_tensor(
                                out=ysb[:], in0=yp, scalar=sc, in1=ysb[:],
                                op0=MULT, op1=ADD,
                            )
                    nc.sync.dma_start(
                        out=y_d[q0 + s * 128:q0 + (s + 1) * 128, :],
                        in_=ysb[:],
                    )

    _split_excess_waits(nc)
    nc.finalize()
    return nc


_NC_CACHE = {}


def _get_nc(t=T):
    if t not in _NC_CACHE:
        _NC_CACHE[t] = build(t)
    return _NC_CACHE[t]


def _make_masks(bf16):
    f = np.arange(128)[None, :]
    p = np.arange(128)[:, None]
    tri = (f < p).astype(np.float32) * NEG     # -30000 where q < k
    zero = np.zeros((128, 512), np.float32)
    full = np.full((128, 128), NEG, np.float32)
    bmask = np.concatenate([tri, zero], axis=1)            # [128, 640]
    bzmask = np.concatenate([full, tri, zero], axis=1)     # [128, 768]
    return bmask.astype(bf16), bzmask.astype(bf16)


def _prep_core_inputs(x_b, w_attn, w_proj, hg, bf16):
    """Host-side shard prep for one core: batch x_b, head group hg (0/1)."""
    h0 = 3 * hg
    q = w_attn[:, 0:C]
    k = w_attn[:, C:2 * C]
    v = w_attn[:, 2 * C:3 * C]
    qcols = lambda h: q[:, h * D:(h + 1) * D]
    kcols = lambda h: k[:, h * D:(h + 1) * D]
    w_q01 = np.concatenate([qcols(h0), qcols(h0 + 1)], axis=1)      # [384,128]
    w_k01 = np.concatenate([kcols(h0), kcols(h0 + 1)], axis=1)
    w_q2 = np.concatenate([qcols(h0 + 2), qcols(h0 + 2)], axis=1)   # dup halves
    w_k2 = np.concatenate([kcols(h0 + 2), kcols(h0 + 2)], axis=1)
    w_v = v[:, h0 * D:(h0 + 3) * D]                                 # [384,192]
    w_o = w_proj[h0 * D:(h0 + 3) * D, :]                            # [192,384]
    bmask, bzmask = _make_masks(bf16)
    return {
        "xT16": np.ascontiguousarray(x_b.T, dtype=bf16),
        "w_q01": np.ascontiguousarray(w_q01.reshape(3, 128, 128), dtype=bf16),
        "w_k01": np.ascontiguousarray(w_k01.reshape(3, 128, 128), dtype=bf16),
        "w_q2": np.ascontiguousarray(w_q2.reshape(3, 128, 128), dtype=bf16),
        "w_k2": np.ascontiguousarray(w_k2.reshape(3, 128, 128), dtype=bf16),
        "w_v": np.ascontiguousarray(w_v.reshape(3, 128, 192), dtype=bf16),
        "w_o": np.ascontiguousarray(w_o.reshape(3, 64, 384), dtype=bf16),
        "bmask": bmask,
        "bzmask": bzmask,
    }


def _build_in_maps(x, w_attn, w_proj):
    import ml_dtypes
    bf16 = ml_dtypes.bfloat16
    in_maps = []
    for core in range(8):
        im = _prep_core_inputs(
            x[core // 2], w_attn, w_proj, core % 2, bf16
        )
        in_maps.append(im)
    return in_maps


def kernel(x, w_attn, w_proj):
    x = np.asarray(x, dtype=np.float32)
    w_attn = np.asarray(w_attn, dtype=np.float32)
    w_proj = np.asarray(w_proj, dtype=np.float32)
    b, t, c = x.shape

    nc = _get_nc(t)
    in_maps = _build_in_maps(x, w_attn, w_proj)

    res = run_bass_kernel_spmd(nc, in_maps, list(range(8)))
    out = np.empty((b, t, c), dtype=np.float32)
    for bb in range(b):
        out[bb] = res.results[2 * bb]["y"] + res.results[2 * bb + 1]["y"]
    return out
